# revision 38
# baseline (speedup 1.0000x reference)
"""Trainium2 Bass kernel for GQA attention (B=2, S=2048, D=2048, H=16, KVH=4).

Sharding: 8 cores = (batch b in {0,1}) x (kv-group g in {0..3}).
Core c = b*4 + g computes q-heads 4g..4g+3 against kv-head g for batch b,
producing a partial output projection res_partial.T = [e=2048, s=2048] in
bf16; host sums the 4 partials per batch.

v2 design notes (vs the f32r baseline at ~410us):
  - TRN2's PE p-states (0.65 -> 1.2 -> 2.4 GHz, 3us continuous ramp) punish
    every stall; the whole structure exists to keep the PE stream gap-free.
  - All matmuls in bf16 (psums f32): full 1 cyc/row at any free size, lower
    power (less 50% utilization throttling), half the DMA/SBUF footprint.
    Error budget ~1% vs the 2e-2 gate.
  - Weights fully resident in SBUF (no per-chunk wq/wo reloads).
  - RMS-norm partition reduce + broadcasts on the PE (tiny ones-matmuls)
    instead of gpsimd (2.5us/op there).
  - Causal masks are precomputed bf16 0/1 tiles applied by DVE multiply
    (affine_select on gpsimd was 0.6us + heavy semaphore cost).
  - exp on ACT over [128,1024] PSUM groups (amortizes the 352-cycle fixed
    overhead); scores/PV/denominator pipelined one group behind so the exp
    latency never blocks the PE.
  - Softmax denominator stays on the PE (ones-column matmul): cheaper there
    than any partition-reduction alternative.
  - Stage D (o-proj) accumulates in PSUM, copied to bf16 SBUF by DVE (2x
    mode) and DMA'd out as bf16 partials.
"""

import sys

sys.path.insert(0, "/opt/trn_rl_repo")

from contextlib import ExitStack

import numpy as np
import ml_dtypes

import concourse.bass as bass
import concourse.tile as tile
from concourse import bacc, mybir
from concourse import bass_utils

B, S, D = 2, 2048, 2048
H, KVH = 16, 4
HD = 128               # head dim
GQ = 4                 # q heads per core
SL = GQ * HD           # 512: q-head slice width per core
NCORES = 8
SC = S // 512          # 4 s-chunks of 512
KC = D // 128          # 16 d-chunks of 128
ROPE_BASE = 10000.0
EPS = 1.1920929e-07
F32 = mybir.dt.float32
F32R = mybir.dt.float32r
BF16 = mybir.dt.bfloat16
AF = mybir.ActivationFunctionType
BF = ml_dtypes.bfloat16

_COMPILED_NC = None
_LAST_IN_MAPS = None


def _build_body(tc):
    nc = tc.nc
    ctx = ExitStack()
    ctx.enter_context(nc.allow_low_precision(reason="bf16 datapath"))

    xT = nc.dram_tensor("xT", [D, S], BF16, kind="ExternalInput").ap()
    wqT = nc.dram_tensor("wqT", [D, SL], BF16, kind="ExternalInput").ap()
    wkT = nc.dram_tensor("wkT", [D, HD], BF16, kind="ExternalInput").ap()
    wvT = nc.dram_tensor("wvT", [D, HD], BF16, kind="ExternalInput").ap()
    woB = nc.dram_tensor("woB", [SL, D], BF16, kind="ExternalInput").ap()
    csd = nc.dram_tensor("csd", [128, S], BF16, kind="ExternalInput").ap()
    snd = nc.dram_tensor("snd", [128, S], BF16, kind="ExternalInput").ap()
    bqkd = nc.dram_tensor("bqkd", [128, GQ + 1], F32, kind="ExternalInput").ap()
    constd = nc.dram_tensor("constd", [257], F32, kind="ExternalInput").ap()
    sced = nc.dram_tensor("sced", [128, 2 * (GQ + 1)], F32, kind="ExternalInput").ap()
    # bf16 consts: [ident(128) | ones_col(1) | masks(4*512)]
    cbd = nc.dram_tensor("cbd", [128, 128 + 1 + 4 * 512], BF16,
                         kind="ExternalInput").ap()
    resT = nc.dram_tensor("resT", [D, S], BF16, kind="ExternalOutput").ap()

    persist = ctx.enter_context(tc.tile_pool(name="persist", bufs=1))
    xpool = ctx.enter_context(tc.tile_pool(name="xpool", bufs=18))
    bpool = ctx.enter_context(tc.tile_pool(name="bpool", bufs=2))
    rowp = ctx.enter_context(tc.tile_pool(name="rowp", bufs=2))
    expp = ctx.enter_context(tc.tile_pool(name="expp", bufs=3))
    otp = ctx.enter_context(tc.tile_pool(name="otp", bufs=2))
    resp = ctx.enter_context(tc.tile_pool(name="resp", bufs=4))
    vtp = ctx.enter_context(tc.tile_pool(name="vtp", bufs=2))
    # PSUM: exactly 8 banks. psS holds projection-pass pairs, score groups,
    # and stage-D accumulators; psO holds rms broadcasts, v-transposes and
    # attention accumulators; psD holds row vectors + recip broadcasts.
    psS = ctx.enter_context(tc.tile_pool(name="psS", bufs=2, space="PSUM"))
    psO = ctx.enter_context(tc.tile_pool(name="psO", bufs=2, space="PSUM"))
    psD = ctx.enter_context(tc.tile_pool(name="psD", bufs=2, space="PSUM"))

    # ---- persistent tiles ----
    cs_sb = persist.tile([128, S], BF16, name="cs_sb")
    sn_sb = persist.tile([128, S], BF16, name="sn_sb")
    wq_sb = {}
    for kq in range(4):
        for pi in range(2):
            wq_sb[(kq, pi)] = persist.tile([128, 4, 256], BF16,
                                           name=f"wq{kq}_{pi}")
    wk_sb = persist.tile([128, KC, HD], BF16, name="wk_sb")
    wv_sb = persist.tile([128, KC, HD], BF16, name="wv_sb")
    wo_sb = [persist.tile([128, GQ, 256], BF16, name=f"wo{e}") for e in range(8)]
    ident = persist.tile([128, 128], BF16, name="ident")
    ones_col = persist.tile([128, 1], BF16, name="ones_col")
    bqcols = persist.tile([128, GQ + 1], F32, name="bqcols")
    sce = persist.tile([128, 2 * (GQ + 1)], F32, name="sce")

    qfin = [persist.tile([128, S], BF16, name=f"qfin{h}") for h in range(GQ)]
    kfin = persist.tile([128, S], BF16, name="kfin")
    v_sb = [persist.tile([128, HD], BF16, name=f"vsb{i}") for i in range(KC)]

    def emit_const_dmas_small():
        # tiny constants needed by the first stage-B chains (~18us in)
        nc.scalar.dma_start(bqcols, bqkd)
        nc.scalar.dma_start(sce, sced)
        nc.scalar.dma_start(ident, cbd[:, 0:128])
        nc.scalar.dma_start(ones_col, cbd[:, 128:129])

    def emit_const_dmas_mid():
        # rope tables, needed by the first stage-B part2 (~25us in)
        nc.scalar.dma_start(cs_sb, csd)
        nc.sync.dma_start(sn_sb, snd)

    def emit_const_dmas_late():
        # needed by stage D only (~160us in)
        for e in range(8):
            nc.scalar.dma_start(
                wo_sb[e],
                bass.AP(tensor=woB.tensor, offset=e * 256,
                        ap=[[D, 128], [128 * D, GQ], [1, 256]]))

    # ================= Stage B: bias, rms-norm, rope (per [128,512] slice) ===
    # Two-level deferral: part1 (psum reads + sumsq matmul + rsqrt) runs one
    # pass behind the projections; part2 (the rsqrt broadcast matmul + rope)
    # runs two passes behind so the PE never waits on the ACT rsqrt.
    pending_b1 = []
    pending_b2 = []

    def stage_b(et, sc, psum_half):
        """et in 0..3 -> q head et;  et == 4 -> k."""
        is_q = et < GQ
        bias_col = bqcols[:, et: et + 1] if is_q else bqcols[:, GQ: GQ + 1]
        state = {}

        def emit1():
            q_raw = bpool.tile([128, 512], BF16, tag="qraw", bufs=3,
                               name=f"qraw{et}_{sc}")
            nc.scalar.add(q_raw, psum_half, bias_col)
            sq = bpool.tile([128, 512], BF16, tag="sq", bufs=2,
                            name=f"sq{et}_{sc}")
            nc.scalar.activation(sq, psum_half, AF.Square, bias=bias_col)
            ssq = psD.tile([1, 512], F32, tag="pD", name=f"ssq{et}_{sc}")
            nc.tensor.matmul(ssq, ones_col, sq, start=True, stop=True)
            # gain & eps folded: g*rsqrt(ss/HD + eps) == rsqrt(ss*A + B),
            # computed as exp(-0.5*ln(ss*A + B)) so the WHOLE kernel stays
            # on the natural_log_exp_and_others ACT table set (the tile
            # scheduler freely interleaves these with stage-C exps; a
            # second table set would ping-pong 1.3us ACT_TABLE_LOADs)
            lns = bpool.tile([1, 512], F32, tag="lns", bufs=3,
                             name=f"lns{et}_{sc}")
            nc.scalar.activation(lns, ssq, AF.Ln,
                                 bias=sce[0:1, GQ + 1 + et: GQ + 2 + et],
                                 scale=sce[0:1, et: et + 1])
            scale_row = bpool.tile([1, 512], F32, tag="srow", bufs=3,
                                   name=f"srow{et}_{sc}")
            nc.scalar.activation(scale_row, lns, AF.Exp, scale=-0.5)
            # rope swap via sbuf->sbuf DMA (sn rows 64..127 hold -sin)
            sw = bpool.tile([128, 512], BF16, tag="sw", bufs=3,
                            name=f"sw{et}_{sc}")
            nc.sync.dma_start(sw[0:64, :], q_raw[64:128, :])
            nc.sync.dma_start(sw[64:128, :], q_raw[0:64, :])
            state["q_raw"] = q_raw
            state["scale_row"] = scale_row
            state["sw"] = sw

        def emit2():
            # scale broadcast on gpsimd: keeps the PE stream and PSUM pools
            # untouched by the rms chain
            bc = bpool.tile([128, 512], F32, tag="bc", bufs=2,
                            name=f"bc{et}_{sc}")
            nc.gpsimd.partition_broadcast(bc, state["scale_row"])
            t1 = bpool.tile([128, 512], BF16, tag="t1", bufs=2,
                            name=f"t1_{et}_{sc}")
            nc.vector.tensor_mul(t1, state["q_raw"],
                                 cs_sb[:, sc * 512: (sc + 1) * 512])
            t2 = bpool.tile([128, 512], BF16, tag="t2", bufs=2,
                            name=f"t2_{et}_{sc}")
            nc.vector.tensor_mul(t2, state["sw"],
                                 sn_sb[:, sc * 512: (sc + 1) * 512])
            nc.vector.tensor_add(t1, t1, t2)
            dst = qfin[et] if is_q else kfin
            nc.vector.tensor_mul(dst[:, sc * 512: (sc + 1) * 512], t1, bc)

        return emit1, emit2

    def flush_b():
        while pending_b2:
            pending_b2.pop(0)()
        while pending_b1:
            e1, e2 = pending_b1.pop(0)
            e1()
            pending_b2.append(e2)

    # ================= Stage A: projections ==================================
    pending_vts = []

    def flush_vts():
        while pending_vts:
            vt, vsc = pending_vts.pop(0)
            for j in range(4):
                stile = vsc * 4 + j
                pst = psO.tile([128, 128], BF16, tag="pO", name=f"pst{stile}")
                nc.tensor.transpose(pst, vt[:, j * 128: (j + 1) * 128], ident)
                nc.vector.tensor_copy(v_sb[stile], pst)

    def emit_xq_dma(xt, sc, kp, eng):
        eng.dma_start(
            xt,
            bass.AP(
                tensor=xT.tensor,
                offset=kp * 256 * S + sc * 512,
                ap=[[S, 128], [128 * S, 2], [1, 512]],
            ),
        )

    xtiles = {}

    def emit_x_dmas(sc, split):
        if sc in xtiles or sc >= SC:
            return
        tl = []
        for kp in range(8):
            xt = xpool.tile([128, 2, 512], BF16, name=f"xq{sc}_{kp}",
                            tag="xq")
            if split and kp == 0:
                # finest-priority first tile so matmuls can begin ASAP
                for c2 in range(2):
                    nc.sync.dma_start(
                        xt[:, c2, :],
                        bass.AP(tensor=xT.tensor, offset=c2 * 128 * S,
                                ap=[[S, 128], [1, 512]]),
                    )
            else:
                emit_xq_dma(xt, sc, kp, nc.sync)
            tl.append(xt)
        xtiles[sc] = tl

    for sc in range(SC):
        flush_vts()
        if sc == 0:
            # startup: kv weights first on scalar (kv pass runs first), then
            # x kp4-7, then wq (needed from pass 2 on)
            nc.scalar.dma_start(
                wk_sb, wkT.rearrange("(kc p) h -> p kc h", p=128))
            nc.scalar.dma_start(
                wv_sb, wvT.rearrange("(kc p) h -> p kc h", p=128))
            emit_x_dmas(0, split=True)
            for kq in range(4):
                for pi in range(2):
                    nc.scalar.dma_start(
                        wq_sb[(kq, pi)],
                        bass.AP(tensor=wqT.tensor,
                                offset=kq * 512 * SL + pi * 256,
                                ap=[[SL, 128], [128 * SL, 4], [1, 256]]),
                    )
        xq = xtiles[sc]

        # passes: (k, vT) first, then q-head pairs (0,1), (2,3)
        for pi, grp in enumerate(((GQ, GQ + 1), (0, 1), (2, 3))):
            ps = psS.tile([128, 1024], F32, tag="pS", name=f"pa{sc}_{pi}")
            for kq in range(4):
                for kc4 in range(4):
                    kc = kq * 4 + kc4
                    start = kc == 0
                    stop = kc == KC - 1
                    xsl = xq[kc // 2][:, kc % 2, :]
                    for ei, et in enumerate(grp):
                        if et < GQ:
                            lhsT = wq_sb[(kq, pi - 1)][:, kc4,
                                                       ei * 128: (ei + 1) * 128]
                        elif et == GQ:
                            lhsT = wk_sb[:, kc, :]
                        else:
                            lhsT = wv_sb[:, kc, :]
                        nc.tensor.matmul(ps[:, ei * 512: (ei + 1) * 512],
                                         lhsT, xsl, start=start, stop=stop)
            if pi == 0:
                if sc == 0:
                    emit_const_dmas_small()
                emit_x_dmas(sc + 1, split=False)
                if sc == 0:
                    emit_const_dmas_mid()
                elif sc == 1:
                    emit_const_dmas_late()
            flush_b()
            for ei, et in enumerate(grp):
                half = ps[:, ei * 512: (ei + 1) * 512]
                if et <= GQ:
                    pending_b1.append(stage_b(et, sc, half))
                else:
                    # vT chunk -> bf16 sbuf; PE-transposes deferred to the
                    # next sc so they never stall the pass-boundary stream
                    vt = vtp.tile([128, 512], BF16, tag="vt", bufs=2,
                                  name=f"vt{sc}")
                    nc.scalar.copy(vt, half)
                    pending_vts.append((vt, sc))

    flush_b()
    flush_b()
    flush_vts()

    # ================= Stage C: attention | Stage D: output proj =============
    # Flat software pipeline over (qc, head, kt-pair-group) work items.
    # PV/denominator drains run ~2 groups behind their exp, interleaved
    # between score matmuls so same-PSUM-bank accumulates stay >=3 apart
    # (back-to-back accumulates into one bank serialize at ~750ns). The
    # normalization chain runs 4-5 items behind (reciprocal on DVE,
    # broadcast on the otherwise-idle gpsimd). Attention psums are
    # allocated lazily at their first drain so pool-buffer reuse never
    # races a not-yet-emitted reader. Stage D for qc is spread as
    # bank-interleaved etg-pairs between the group items of qc+1.

    def groups_for(qc):
        diag = [(4 * qc, 4 * qc + 1), (4 * qc + 2, 4 * qc + 3)]
        off = [(k, k + 1) for k in range(0, 4 * qc, 2)]
        return diag + off

    seq = []
    for qc in range(SC):
        gl = groups_for(qc)
        gitems = [(qc, h, pair, gi == 0, gi == len(gl) - 1)
                  for h in range(GQ) for gi, pair in enumerate(gl)]
        for idx, it in enumerate(gitems):
            seq.append(("g",) + it)
            if qc >= 1 and idx in (5, 7, 9, 11):
                seq.append(("d", qc - 1, idx - 5))
    seq.append(("flushdef",))
    for dp in range(4):
        seq.append(("d", SC - 1, 2 * dp))

    psum_o = {}
    psum_d = {}
    ot_tiles = {}
    pvq = []       # FIFO of half-group PV/denominator drains
    norms = []     # (due_step, fn), dues monotone

    def drain_pv(keep):
        while len(pvq) > keep:
            pvq.pop(0)()

    def flush_norms(s):
        while norms and norms[0][0] <= s:
            norms.pop(0)[1]()

    rf_rows = {}

    def make_pv_half(qc, h, kt, ex, j, st, sp):
        def emit():
            if st:
                psum_o[(qc, h)] = psO.tile([128, 512], F32, tag="pO",
                                           name=f"pso{qc}_{h}")
                psum_d[(qc, h)] = psD.tile([1, 512], F32, tag="pD",
                                           name=f"psd_{qc}_{h}")
            exsl = ex[:, j * 512: (j + 1) * 512]
            nc.tensor.matmul(psum_o[(qc, h)], v_sb[kt], exsl,
                             start=st, stop=sp)
            nc.tensor.matmul(psum_d[(qc, h)], ones_col, exsl,
                             start=st, stop=sp)
        return emit

    def make_norm_a(qc, h):
        def emit():
            rf = rowp.tile([1, 512], F32, tag="rf", bufs=2,
                           name=f"rf{qc}_{h}")
            nc.vector.reciprocal_approx_fast(rf, psum_d[(qc, h)])
            rf_rows[(qc, h)] = rf
        return emit

    def make_norm_b(qc, h):
        def emit():
            rb = bpool.tile([128, 512], F32, tag="rb", bufs=2,
                            name=f"rb{qc}_{h}")
            nc.gpsimd.partition_broadcast(rb, rf_rows[(qc, h)])
            ot = otp.tile([128, 512], BF16, tag=f"ot{h}", name=f"ot{qc}_{h}")
            nc.vector.tensor_mul(ot, psum_o[(qc, h)], rb)
            ot_tiles[(qc, h)] = ot
        return emit

    def emit_stage_d_pair(qc, etg0, tail=False):
        # two etg accumulators live at once; matmuls interleave the four
        # PSUM banks so same-bank accumulates are >=3 apart
        pss = [psS.tile([128, 1024], F32, tag="pS", name=f"psd{qc}_{e}")
               for e in (etg0, etg0 + 1)]
        for h in range(GQ):
            for e2 in range(2):
                for ei, etg in enumerate((etg0, etg0 + 1)):
                    nc.tensor.matmul(
                        pss[ei][:, e2 * 512: (e2 + 1) * 512],
                        wo_sb[etg][:, h, e2 * 128: (e2 + 1) * 128],
                        ot_tiles[(qc, h)],
                        start=(h == 0), stop=(h == GQ - 1),
                    )
        for ei, etg in enumerate((etg0, etg0 + 1)):
            r = resp.tile([128, 2, 512], BF16, tag="res", name=f"r{qc}_{etg}")
            for e2 in range(2):
                # at the tail there are no more exps: use the idle ACT for
                # half the PSUM->SBUF copies so psS bufs recycle faster
                if tail and e2 == 0:
                    nc.scalar.copy(r[:, e2, :],
                                   pss[ei][:, e2 * 512: (e2 + 1) * 512])
                else:
                    nc.vector.tensor_copy(r[:, e2, :],
                                          pss[ei][:, e2 * 512: (e2 + 1) * 512])
            eng = nc.sync
            eng.dma_start(
                bass.AP(tensor=resT.tensor,
                        offset=etg * 2 * 128 * S + qc * 512,
                        ap=[[S, 128], [128 * S, 2], [1, 512]]),
                r,
            )

    step = 0
    for it in seq:
        if it[0] == "g":
            _, qc, h, pair, first, last = it
            flush_norms(step)
            ps = psS.tile([128, 1024], F32, tag="pS",
                          name=f"pss{qc}_{h}_{pair[0]}")
            qsl = qfin[h][:, qc * 512: (qc + 1) * 512]
            ex = expp.tile([128, 1024], BF16, tag="exp",
                           name=f"ex{qc}_{h}_{pair[0]}")
            for j, kt in enumerate(pair):
                nc.tensor.matmul(ps[:, j * 512: (j + 1) * 512],
                                 kfin[:, kt * 128: (kt + 1) * 128], qsl,
                                 start=True, stop=True)
                # keep a 2-group backlog of PV drains between score matmuls
                drain_pv(4 - j)
            nc.scalar.activation(ex, ps, AF.Exp)
            t0 = pair[0] - 4 * qc
            if t0 >= 0:
                # diagonal pair: causal mask (keep q >= k) on gpsimd so the
                # DVE queue never gates the PV chain
                for j in range(2):
                    nc.gpsimd.affine_select(
                        out=ex[:, j * 512: (j + 1) * 512],
                        in_=ex[:, j * 512: (j + 1) * 512],
                        pattern=[[1, 512]],
                        compare_op=mybir.AluOpType.is_ge,
                        fill=0.0,
                        base=-128 * (t0 + j),
                        channel_multiplier=-1,
                    )
            for j, kt in enumerate(pair):
                pvq.append(make_pv_half(qc, h, kt, ex, j,
                                        first and j == 0, last and j == 1))
            if last:
                norms.append((step + 4, make_norm_a(qc, h)))
                norms.append((step + 5, make_norm_b(qc, h)))
            step += 1
        elif it[0] == "d":
            emit_stage_d_pair(it[1], it[2], tail=(it[1] == SC - 1))
        else:  # flushdef
            drain_pv(0)
            flush_norms(10 ** 9)

    ctx.close()


def _build():
    global _COMPILED_NC
    if _COMPILED_NC is not None:
        return _COMPILED_NC
    nc = bacc.Bacc("TRN2", target_bir_lowering=False, debug=False,
                   num_devices=NCORES)
    with tile.TileContext(nc) as tc:
        _build_body(tc)
    nc.compile()
    _COMPILED_NC = nc
    return nc


def _rope_tables():
    inv_freq = 1.0 / (ROPE_BASE ** (np.arange(0, HD, 2, dtype=np.float64) / HD))
    t = np.arange(S, dtype=np.float64)
    freqs = np.outer(t, inv_freq)          # [S, 64]
    cos = np.cos(freqs).T.astype(np.float32)   # [64, S]
    sin = np.sin(freqs).T.astype(np.float32)
    cs = np.concatenate([cos, cos], axis=0)    # [128, S]
    sn = np.concatenate([sin, -sin], axis=0)   # [128, S] (bottom half negated)
    return cs.astype(BF), sn.astype(BF)


def _mask_tiles():
    # maskt[p, t*512 + j] = 1 if causal keeps (k_local=p+128t, q_local=j)
    p = np.arange(128)[:, None]
    j = np.arange(512)[None, :]
    cols = [(j >= p + 128 * t).astype(np.float32) for t in range(4)]
    return np.concatenate(cols, axis=1).astype(BF)


def kernel(x, Wq, bq, Wk, bk, Wv, bv, Wo, bo, q_gain):
    x = np.asarray(x, np.float32)
    Wq = np.asarray(Wq, np.float32)
    bq = np.asarray(bq, np.float32)
    Wk = np.asarray(Wk, np.float32)
    bk = np.asarray(bk, np.float32)
    Wv = np.asarray(Wv, np.float32)
    bv = np.asarray(bv, np.float32)
    Wo = np.asarray(Wo, np.float32)
    bo = np.asarray(bo, np.float32)
    q_gain = np.asarray(q_gain, np.float32)

    cs, sn = _rope_tables()
    const_arr = np.concatenate([
        np.ones(128, np.float32),
        np.full(128, float(HD) ** -0.5, np.float32),
        np.array([EPS], np.float32),
    ])
    cb = np.concatenate([
        np.eye(128, dtype=np.float32).astype(BF),
        np.ones((128, 1), np.float32).astype(BF),
        _mask_tiles(),
    ], axis=1)

    def sced_arr(g):
        gg = np.concatenate([g.astype(np.float64), [HD ** -0.5]])
        a = 1.0 / (HD * gg * gg)
        b = EPS / (gg * gg)
        row = np.concatenate([a, b]).astype(np.float32)   # [2*(GQ+1)]
        return np.ascontiguousarray(np.broadcast_to(row, (128, row.size)))

    in_maps = []
    for c in range(NCORES):
        b, g = divmod(c, KVH)
        sl = slice(g * SL, (g + 1) * SL)
        hs = slice(g * HD, (g + 1) * HD)
        in_maps.append({
            "xT": np.ascontiguousarray(x[b].T.astype(BF)),
            "wqT": np.ascontiguousarray(Wq[sl, :].T.astype(BF)),
            "wkT": np.ascontiguousarray(Wk[hs, :].T.astype(BF)),
            "wvT": np.ascontiguousarray(Wv[hs, :].T.astype(BF)),
            "woB": np.ascontiguousarray(Wo[:, sl].T.astype(BF)),
            "csd": cs, "snd": sn,
            "bqkd": np.ascontiguousarray(np.concatenate(
                [bq[sl].reshape(GQ, HD).T, bk[hs].reshape(1, HD).T], axis=1)),
            "sced": sced_arr(q_gain[g * GQ: (g + 1) * GQ]),
            "constd": const_arr,
            "cbd": cb,
        })

    global _LAST_IN_MAPS
    _LAST_IN_MAPS = in_maps
    nc = _build()
    res = bass_utils.run_bass_kernel_spmd(nc, in_maps, core_ids=list(range(NCORES)))

    # v-bias and o-bias folded on host: attention rows sum to 1, so +bv
    # passes through to O exactly; res += bv_rep @ Wo.T + bo.
    bv_rep = np.repeat(bv.reshape(KVH, HD), H // KVH, axis=0).reshape(-1)
    host_const = (Wo @ bv_rep + bo).astype(np.float32)

    out = np.zeros((B, S, D), np.float32)
    for c in range(NCORES):
        b = c // KVH
        out[b] += res.results[c]["resT"].astype(np.float32).T
    out += host_const[None, None, :]
    return out


# revision 40
# speedup vs baseline: 1.0176x; 1.0176x over previous
"""Trainium2 Bass kernel for GQA attention (B=2, S=2048, D=2048, H=16, KVH=4).

Sharding: 8 cores = (batch b in {0,1}) x (kv-group g in {0..3}).
Core c = b*4 + g computes q-heads 4g..4g+3 against kv-head g for batch b,
producing a partial output projection res_partial.T = [e=2048, s=2048] in
bf16; host sums the 4 partials per batch.

v2 design notes (vs the f32r baseline at ~410us):
  - TRN2's PE p-states (0.65 -> 1.2 -> 2.4 GHz, 3us continuous ramp) punish
    every stall; the whole structure exists to keep the PE stream gap-free.
  - All matmuls in bf16 (psums f32): full 1 cyc/row at any free size, lower
    power (less 50% utilization throttling), half the DMA/SBUF footprint.
    Error budget ~1% vs the 2e-2 gate.
  - Weights fully resident in SBUF (no per-chunk wq/wo reloads).
  - RMS-norm partition reduce + broadcasts on the PE (tiny ones-matmuls)
    instead of gpsimd (2.5us/op there).
  - Causal masks are precomputed bf16 0/1 tiles applied by DVE multiply
    (affine_select on gpsimd was 0.6us + heavy semaphore cost).
  - exp on ACT over [128,1024] PSUM groups (amortizes the 352-cycle fixed
    overhead); scores/PV/denominator pipelined one group behind so the exp
    latency never blocks the PE.
  - Softmax denominator stays on the PE (ones-column matmul): cheaper there
    than any partition-reduction alternative.
  - Stage D (o-proj) accumulates in PSUM, copied to bf16 SBUF by DVE (2x
    mode) and DMA'd out as bf16 partials.
"""

import sys

sys.path.insert(0, "/opt/trn_rl_repo")

from contextlib import ExitStack

import numpy as np
import ml_dtypes

import concourse.bass as bass
import concourse.tile as tile
from concourse import bacc, mybir
from concourse import bass_utils

B, S, D = 2, 2048, 2048
H, KVH = 16, 4
HD = 128               # head dim
GQ = 4                 # q heads per core
SL = GQ * HD           # 512: q-head slice width per core
NCORES = 8
SC = S // 512          # 4 s-chunks of 512
KC = D // 128          # 16 d-chunks of 128
ROPE_BASE = 10000.0
EPS = 1.1920929e-07
F32 = mybir.dt.float32
F32R = mybir.dt.float32r
BF16 = mybir.dt.bfloat16
AF = mybir.ActivationFunctionType
BF = ml_dtypes.bfloat16

_COMPILED_NC = None
_LAST_IN_MAPS = None


def _build_body(tc):
    nc = tc.nc
    ctx = ExitStack()
    ctx.enter_context(nc.allow_low_precision(reason="bf16 datapath"))

    xT = nc.dram_tensor("xT", [D, S], BF16, kind="ExternalInput").ap()
    wqT = nc.dram_tensor("wqT", [D, SL], BF16, kind="ExternalInput").ap()
    wkT = nc.dram_tensor("wkT", [D, HD], BF16, kind="ExternalInput").ap()
    wvT = nc.dram_tensor("wvT", [D, HD], BF16, kind="ExternalInput").ap()
    woB = nc.dram_tensor("woB", [SL, D], BF16, kind="ExternalInput").ap()
    csd = nc.dram_tensor("csd", [128, S], BF16, kind="ExternalInput").ap()
    snd = nc.dram_tensor("snd", [128, S], BF16, kind="ExternalInput").ap()
    bqkd = nc.dram_tensor("bqkd", [128, GQ + 1], F32, kind="ExternalInput").ap()
    constd = nc.dram_tensor("constd", [257], F32, kind="ExternalInput").ap()
    sced = nc.dram_tensor("sced", [128, 2 * (GQ + 1)], F32, kind="ExternalInput").ap()
    # bf16 consts: [ident(128) | ones_col(1) | masks(4*512)]
    cbd = nc.dram_tensor("cbd", [128, 128 + 1 + 4 * 512], BF16,
                         kind="ExternalInput").ap()
    resT = nc.dram_tensor("resT", [D, S], BF16, kind="ExternalOutput").ap()

    persist = ctx.enter_context(tc.tile_pool(name="persist", bufs=1))
    xpool = ctx.enter_context(tc.tile_pool(name="xpool", bufs=18))
    bpool = ctx.enter_context(tc.tile_pool(name="bpool", bufs=2))
    rowp = ctx.enter_context(tc.tile_pool(name="rowp", bufs=2))
    expp = ctx.enter_context(tc.tile_pool(name="expp", bufs=3))
    otp = ctx.enter_context(tc.tile_pool(name="otp", bufs=2))
    resp = ctx.enter_context(tc.tile_pool(name="resp", bufs=4))
    vtp = ctx.enter_context(tc.tile_pool(name="vtp", bufs=2))
    # PSUM: exactly 8 banks. psS holds projection-pass pairs, score groups,
    # and stage-D accumulators; psO holds rms broadcasts, v-transposes and
    # attention accumulators; psD holds row vectors + recip broadcasts.
    psS = ctx.enter_context(tc.tile_pool(name="psS", bufs=2, space="PSUM"))
    psO = ctx.enter_context(tc.tile_pool(name="psO", bufs=2, space="PSUM"))
    psD = ctx.enter_context(tc.tile_pool(name="psD", bufs=2, space="PSUM"))

    # ---- persistent tiles ----
    cs_sb = persist.tile([128, S], BF16, name="cs_sb")
    sn_sb = persist.tile([128, S], BF16, name="sn_sb")
    wq_sb = {}
    for kq in range(4):
        for pi in range(2):
            wq_sb[(kq, pi)] = persist.tile([128, 4, 256], BF16,
                                           name=f"wq{kq}_{pi}")
    wk_sb = persist.tile([128, KC, HD], BF16, name="wk_sb")
    wv_sb = persist.tile([128, KC, HD], BF16, name="wv_sb")
    wo_sb = [persist.tile([128, GQ, 256], BF16, name=f"wo{e}") for e in range(8)]
    ident = persist.tile([128, 128], BF16, name="ident")
    ones_col = persist.tile([128, 1], BF16, name="ones_col")
    bqcols = persist.tile([128, GQ + 1], F32, name="bqcols")
    sce = persist.tile([128, 2 * (GQ + 1)], F32, name="sce")

    qfin = [persist.tile([128, S], BF16, name=f"qfin{h}") for h in range(GQ)]
    kfin = persist.tile([128, S], BF16, name="kfin")
    v_sb = [persist.tile([128, HD], BF16, name=f"vsb{i}") for i in range(KC)]

    def emit_const_dmas_small():
        # tiny constants needed by the first stage-B chains (~18us in)
        nc.scalar.dma_start(bqcols, bqkd)
        nc.scalar.dma_start(sce, sced)
        nc.scalar.dma_start(ident, cbd[:, 0:128])
        nc.scalar.dma_start(ones_col, cbd[:, 128:129])

    def emit_const_dmas_mid():
        # rope tables, needed by the first stage-B part2 (~25us in)
        nc.scalar.dma_start(cs_sb, csd)
        nc.sync.dma_start(sn_sb, snd)

    def emit_const_dmas_late():
        # needed by stage D only (~160us in)
        for e in range(8):
            nc.scalar.dma_start(
                wo_sb[e],
                bass.AP(tensor=woB.tensor, offset=e * 256,
                        ap=[[D, 128], [128 * D, GQ], [1, 256]]))

    # ================= Stage B: bias, rms-norm, rope (per [128,512] slice) ===
    # Two-level deferral: part1 (psum reads + sumsq matmul + rsqrt) runs one
    # pass behind the projections; part2 (the rsqrt broadcast matmul + rope)
    # runs two passes behind so the PE never waits on the ACT rsqrt.
    pending_b1 = []
    pending_b2 = []

    def stage_b(et, sc, psum_half):
        """et in 0..3 -> q head et;  et == 4 -> k."""
        is_q = et < GQ
        bias_col = bqcols[:, et: et + 1] if is_q else bqcols[:, GQ: GQ + 1]
        state = {}

        def emit1a():
            q_raw = bpool.tile([128, 512], BF16, tag="qraw", bufs=3,
                               name=f"qraw{et}_{sc}")
            nc.scalar.add(q_raw, psum_half, bias_col)
            sq = bpool.tile([128, 512], BF16, tag="sq", bufs=2,
                            name=f"sq{et}_{sc}")
            nc.scalar.activation(sq, psum_half, AF.Square, bias=bias_col)
            ssq = psD.tile([1, 512], F32, tag="pD", name=f"ssq{et}_{sc}")
            nc.tensor.matmul(ssq, ones_col, sq, start=True, stop=True)
            # rope swap via sbuf->sbuf DMA (sn rows 64..127 hold -sin)
            sw = bpool.tile([128, 512], BF16, tag="sw", bufs=3,
                            name=f"sw{et}_{sc}")
            nc.sync.dma_start(sw[0:64, :], q_raw[64:128, :])
            nc.sync.dma_start(sw[64:128, :], q_raw[0:64, :])
            state["q_raw"] = q_raw
            state["sw"] = sw
            state["ssq"] = ssq

        def emit1b():
            # gain & eps folded: g*rsqrt(ss/HD + eps) == rsqrt(ss*A + B)
            scale_row = bpool.tile([1, 512], F32, tag="srow", bufs=3,
                                   name=f"srow{et}_{sc}")
            nc.scalar.activation(scale_row, state["ssq"],
                                 AF.Abs_reciprocal_sqrt,
                                 bias=sce[0:1, GQ + 1 + et: GQ + 2 + et],
                                 scale=sce[0:1, et: et + 1])
            state["scale_row"] = scale_row

        def emit2():
            # scale broadcast on gpsimd: keeps the PE stream and PSUM pools
            # untouched by the rms chain
            bc = bpool.tile([128, 512], F32, tag="bc", bufs=2,
                            name=f"bc{et}_{sc}")
            nc.gpsimd.partition_broadcast(bc, state["scale_row"])
            t1 = bpool.tile([128, 512], BF16, tag="t1", bufs=2,
                            name=f"t1_{et}_{sc}")
            nc.vector.tensor_mul(t1, state["q_raw"],
                                 cs_sb[:, sc * 512: (sc + 1) * 512])
            t2 = bpool.tile([128, 512], BF16, tag="t2", bufs=2,
                            name=f"t2_{et}_{sc}")
            nc.vector.tensor_mul(t2, state["sw"],
                                 sn_sb[:, sc * 512: (sc + 1) * 512])
            nc.vector.tensor_add(t1, t1, t2)
            dst = qfin[et] if is_q else kfin
            nc.vector.tensor_mul(dst[:, sc * 512: (sc + 1) * 512], t1, bc)

        return emit1, emit2

    def flush_b():
        while pending_b2:
            pending_b2.pop(0)()
        while pending_b1:
            e1, e2 = pending_b1.pop(0)
            e1()
            pending_b2.append(e2)

    # ================= Stage A: projections ==================================
    pending_vts = []

    def flush_vts():
        while pending_vts:
            vt, vsc = pending_vts.pop(0)
            for j in range(4):
                stile = vsc * 4 + j
                pst = psO.tile([128, 128], BF16, tag="pO", name=f"pst{stile}")
                nc.tensor.transpose(pst, vt[:, j * 128: (j + 1) * 128], ident)
                nc.vector.tensor_copy(v_sb[stile], pst)

    def emit_xq_dma(xt, sc, kp, eng):
        eng.dma_start(
            xt,
            bass.AP(
                tensor=xT.tensor,
                offset=kp * 256 * S + sc * 512,
                ap=[[S, 128], [128 * S, 2], [1, 512]],
            ),
        )

    xtiles = {}

    def emit_x_dmas(sc, split):
        if sc in xtiles or sc >= SC:
            return
        tl = []
        for kp in range(8):
            xt = xpool.tile([128, 2, 512], BF16, name=f"xq{sc}_{kp}",
                            tag="xq")
            if split and kp == 0:
                # finest-priority first tile so matmuls can begin ASAP
                for c2 in range(2):
                    nc.sync.dma_start(
                        xt[:, c2, :],
                        bass.AP(tensor=xT.tensor, offset=c2 * 128 * S,
                                ap=[[S, 128], [1, 512]]),
                    )
            else:
                emit_xq_dma(xt, sc, kp, nc.sync)
            tl.append(xt)
        xtiles[sc] = tl

    for sc in range(SC):
        flush_vts()
        if sc == 0:
            # startup: kv weights first on scalar (kv pass runs first), then
            # x kp4-7, then wq (needed from pass 2 on)
            nc.scalar.dma_start(
                wk_sb, wkT.rearrange("(kc p) h -> p kc h", p=128))
            nc.scalar.dma_start(
                wv_sb, wvT.rearrange("(kc p) h -> p kc h", p=128))
            emit_x_dmas(0, split=True)
            for kq in range(4):
                for pi in range(2):
                    nc.scalar.dma_start(
                        wq_sb[(kq, pi)],
                        bass.AP(tensor=wqT.tensor,
                                offset=kq * 512 * SL + pi * 256,
                                ap=[[SL, 128], [128 * SL, 4], [1, 256]]),
                    )
        xq = xtiles[sc]

        # passes: (k, vT) first, then q-head pairs (0,1), (2,3)
        for pi, grp in enumerate(((GQ, GQ + 1), (0, 1), (2, 3))):
            ps = psS.tile([128, 1024], F32, tag="pS", name=f"pa{sc}_{pi}")
            for kq in range(4):
                for kc4 in range(4):
                    kc = kq * 4 + kc4
                    start = kc == 0
                    stop = kc == KC - 1
                    xsl = xq[kc // 2][:, kc % 2, :]
                    for ei, et in enumerate(grp):
                        if et < GQ:
                            lhsT = wq_sb[(kq, pi - 1)][:, kc4,
                                                       ei * 128: (ei + 1) * 128]
                        elif et == GQ:
                            lhsT = wk_sb[:, kc, :]
                        else:
                            lhsT = wv_sb[:, kc, :]
                        nc.tensor.matmul(ps[:, ei * 512: (ei + 1) * 512],
                                         lhsT, xsl, start=start, stop=stop)
            if pi == 0:
                if sc == 0:
                    emit_const_dmas_small()
                emit_x_dmas(sc + 1, split=False)
                if sc == 0:
                    emit_const_dmas_mid()
                elif sc == 1:
                    emit_const_dmas_late()
            flush_b()
            for ei, et in enumerate(grp):
                half = ps[:, ei * 512: (ei + 1) * 512]
                if et <= GQ:
                    pending_b1.append(stage_b(et, sc, half))
                else:
                    # vT chunk -> bf16 sbuf; PE-transposes deferred to the
                    # next sc so they never stall the pass-boundary stream
                    vt = vtp.tile([128, 512], BF16, tag="vt", bufs=2,
                                  name=f"vt{sc}")
                    nc.scalar.copy(vt, half)
                    pending_vts.append((vt, sc))

    flush_b()
    flush_b()
    flush_vts()

    # ================= Stage C: attention | Stage D: output proj =============
    # Flat software pipeline over (qc, head, kt-pair-group) work items.
    # PV/denominator drains run ~2 groups behind their exp, interleaved
    # between score matmuls so same-PSUM-bank accumulates stay >=3 apart
    # (back-to-back accumulates into one bank serialize at ~750ns). The
    # normalization chain runs 4-5 items behind (reciprocal on DVE,
    # broadcast on the otherwise-idle gpsimd). Attention psums are
    # allocated lazily at their first drain so pool-buffer reuse never
    # races a not-yet-emitted reader. Stage D for qc is spread as
    # bank-interleaved etg-pairs between the group items of qc+1.

    def groups_for(qc):
        diag = [(4 * qc, 4 * qc + 1), (4 * qc + 2, 4 * qc + 3)]
        off = [(k, k + 1) for k in range(0, 4 * qc, 2)]
        return diag + off

    seq = []
    for qc in range(SC):
        gl = groups_for(qc)
        gitems = [(qc, h, pair, gi == 0, gi == len(gl) - 1)
                  for h in range(GQ) for gi, pair in enumerate(gl)]
        for idx, it in enumerate(gitems):
            seq.append(("g",) + it)
            if qc >= 1 and idx in (5, 7, 9, 11):
                seq.append(("d", qc - 1, idx - 5))
    seq.append(("flushdef",))
    for dp in range(4):
        seq.append(("d", SC - 1, 2 * dp))

    psum_o = {}
    psum_d = {}
    ot_tiles = {}
    pvq = []       # FIFO of half-group PV/denominator drains
    norms = []     # (due_step, fn), dues monotone

    def drain_pv(keep):
        while len(pvq) > keep:
            pvq.pop(0)()

    def flush_norms(s):
        while norms and norms[0][0] <= s:
            norms.pop(0)[1]()

    rf_rows = {}

    def make_pv_half(qc, h, kt, ex, j, st, sp):
        def emit():
            if st:
                psum_o[(qc, h)] = psO.tile([128, 512], F32, tag="pO",
                                           name=f"pso{qc}_{h}")
                psum_d[(qc, h)] = psD.tile([1, 512], F32, tag="pD",
                                           name=f"psd_{qc}_{h}")
            exsl = ex[:, j * 512: (j + 1) * 512]
            nc.tensor.matmul(psum_o[(qc, h)], v_sb[kt], exsl,
                             start=st, stop=sp)
            nc.tensor.matmul(psum_d[(qc, h)], ones_col, exsl,
                             start=st, stop=sp)
        return emit

    def make_norm_a(qc, h):
        def emit():
            rf = rowp.tile([1, 512], F32, tag="rf", bufs=2,
                           name=f"rf{qc}_{h}")
            nc.vector.reciprocal_approx_fast(rf, psum_d[(qc, h)])
            rf_rows[(qc, h)] = rf
        return emit

    def make_norm_b(qc, h):
        def emit():
            rb = bpool.tile([128, 512], F32, tag="rb", bufs=2,
                            name=f"rb{qc}_{h}")
            nc.gpsimd.partition_broadcast(rb, rf_rows[(qc, h)])
            ot = otp.tile([128, 512], BF16, tag=f"ot{h}", name=f"ot{qc}_{h}")
            nc.vector.tensor_mul(ot, psum_o[(qc, h)], rb)
            ot_tiles[(qc, h)] = ot
        return emit

    def emit_stage_d_pair(qc, etg0, tail=False):
        # two etg accumulators live at once; matmuls interleave the four
        # PSUM banks so same-bank accumulates are >=3 apart
        pss = [psS.tile([128, 1024], F32, tag="pS", name=f"psd{qc}_{e}")
               for e in (etg0, etg0 + 1)]
        for h in range(GQ):
            for e2 in range(2):
                for ei, etg in enumerate((etg0, etg0 + 1)):
                    nc.tensor.matmul(
                        pss[ei][:, e2 * 512: (e2 + 1) * 512],
                        wo_sb[etg][:, h, e2 * 128: (e2 + 1) * 128],
                        ot_tiles[(qc, h)],
                        start=(h == 0), stop=(h == GQ - 1),
                    )
        for ei, etg in enumerate((etg0, etg0 + 1)):
            r = resp.tile([128, 2, 512], BF16, tag="res", name=f"r{qc}_{etg}")
            for e2 in range(2):
                # at the tail there are no more exps: use the idle ACT for
                # half the PSUM->SBUF copies so psS bufs recycle faster
                if tail and e2 == 0:
                    nc.scalar.copy(r[:, e2, :],
                                   pss[ei][:, e2 * 512: (e2 + 1) * 512])
                else:
                    nc.vector.tensor_copy(r[:, e2, :],
                                          pss[ei][:, e2 * 512: (e2 + 1) * 512])
            eng = nc.sync
            eng.dma_start(
                bass.AP(tensor=resT.tensor,
                        offset=etg * 2 * 128 * S + qc * 512,
                        ap=[[S, 128], [128 * S, 2], [1, 512]]),
                r,
            )

    step = 0
    for it in seq:
        if it[0] == "g":
            _, qc, h, pair, first, last = it
            flush_norms(step)
            ps = psS.tile([128, 1024], F32, tag="pS",
                          name=f"pss{qc}_{h}_{pair[0]}")
            qsl = qfin[h][:, qc * 512: (qc + 1) * 512]
            ex = expp.tile([128, 1024], BF16, tag="exp",
                           name=f"ex{qc}_{h}_{pair[0]}")
            for j, kt in enumerate(pair):
                nc.tensor.matmul(ps[:, j * 512: (j + 1) * 512],
                                 kfin[:, kt * 128: (kt + 1) * 128], qsl,
                                 start=True, stop=True)
                # keep a 2-group backlog of PV drains between score matmuls
                drain_pv(4 - j)
            nc.scalar.activation(ex, ps, AF.Exp)
            t0 = pair[0] - 4 * qc
            if t0 >= 0:
                # diagonal pair: causal mask (keep q >= k) on gpsimd so the
                # DVE queue never gates the PV chain
                for j in range(2):
                    nc.gpsimd.affine_select(
                        out=ex[:, j * 512: (j + 1) * 512],
                        in_=ex[:, j * 512: (j + 1) * 512],
                        pattern=[[1, 512]],
                        compare_op=mybir.AluOpType.is_ge,
                        fill=0.0,
                        base=-128 * (t0 + j),
                        channel_multiplier=-1,
                    )
            for j, kt in enumerate(pair):
                pvq.append(make_pv_half(qc, h, kt, ex, j,
                                        first and j == 0, last and j == 1))
            if last:
                norms.append((step + 4, make_norm_a(qc, h)))
                norms.append((step + 5, make_norm_b(qc, h)))
            step += 1
        elif it[0] == "d":
            emit_stage_d_pair(it[1], it[2], tail=(it[1] == SC - 1))
        else:  # flushdef
            drain_pv(0)
            flush_norms(10 ** 9)

    ctx.close()


def _build():
    global _COMPILED_NC
    if _COMPILED_NC is not None:
        return _COMPILED_NC
    nc = bacc.Bacc("TRN2", target_bir_lowering=False, debug=False,
                   num_devices=NCORES)
    with tile.TileContext(nc) as tc:
        _build_body(tc)
    nc.compile()
    _COMPILED_NC = nc
    return nc


def _rope_tables():
    inv_freq = 1.0 / (ROPE_BASE ** (np.arange(0, HD, 2, dtype=np.float64) / HD))
    t = np.arange(S, dtype=np.float64)
    freqs = np.outer(t, inv_freq)          # [S, 64]
    cos = np.cos(freqs).T.astype(np.float32)   # [64, S]
    sin = np.sin(freqs).T.astype(np.float32)
    cs = np.concatenate([cos, cos], axis=0)    # [128, S]
    sn = np.concatenate([sin, -sin], axis=0)   # [128, S] (bottom half negated)
    return cs.astype(BF), sn.astype(BF)


def _mask_tiles():
    # maskt[p, t*512 + j] = 1 if causal keeps (k_local=p+128t, q_local=j)
    p = np.arange(128)[:, None]
    j = np.arange(512)[None, :]
    cols = [(j >= p + 128 * t).astype(np.float32) for t in range(4)]
    return np.concatenate(cols, axis=1).astype(BF)


def kernel(x, Wq, bq, Wk, bk, Wv, bv, Wo, bo, q_gain):
    x = np.asarray(x, np.float32)
    Wq = np.asarray(Wq, np.float32)
    bq = np.asarray(bq, np.float32)
    Wk = np.asarray(Wk, np.float32)
    bk = np.asarray(bk, np.float32)
    Wv = np.asarray(Wv, np.float32)
    bv = np.asarray(bv, np.float32)
    Wo = np.asarray(Wo, np.float32)
    bo = np.asarray(bo, np.float32)
    q_gain = np.asarray(q_gain, np.float32)

    cs, sn = _rope_tables()
    const_arr = np.concatenate([
        np.ones(128, np.float32),
        np.full(128, float(HD) ** -0.5, np.float32),
        np.array([EPS], np.float32),
    ])
    cb = np.concatenate([
        np.eye(128, dtype=np.float32).astype(BF),
        np.ones((128, 1), np.float32).astype(BF),
        _mask_tiles(),
    ], axis=1)

    def sced_arr(g):
        gg = np.concatenate([g.astype(np.float64), [HD ** -0.5]])
        a = 1.0 / (HD * gg * gg)
        b = EPS / (gg * gg)
        row = np.concatenate([a, b]).astype(np.float32)   # [2*(GQ+1)]
        return np.ascontiguousarray(np.broadcast_to(row, (128, row.size)))

    in_maps = []
    for c in range(NCORES):
        b, g = divmod(c, KVH)
        sl = slice(g * SL, (g + 1) * SL)
        hs = slice(g * HD, (g + 1) * HD)
        in_maps.append({
            "xT": np.ascontiguousarray(x[b].T.astype(BF)),
            "wqT": np.ascontiguousarray(Wq[sl, :].T.astype(BF)),
            "wkT": np.ascontiguousarray(Wk[hs, :].T.astype(BF)),
            "wvT": np.ascontiguousarray(Wv[hs, :].T.astype(BF)),
            "woB": np.ascontiguousarray(Wo[:, sl].T.astype(BF)),
            "csd": cs, "snd": sn,
            "bqkd": np.ascontiguousarray(np.concatenate(
                [bq[sl].reshape(GQ, HD).T, bk[hs].reshape(1, HD).T], axis=1)),
            "sced": sced_arr(q_gain[g * GQ: (g + 1) * GQ]),
            "constd": const_arr,
            "cbd": cb,
        })

    global _LAST_IN_MAPS
    _LAST_IN_MAPS = in_maps
    nc = _build()
    res = bass_utils.run_bass_kernel_spmd(nc, in_maps, core_ids=list(range(NCORES)))

    # v-bias and o-bias folded on host: attention rows sum to 1, so +bv
    # passes through to O exactly; res += bv_rep @ Wo.T + bo.
    bv_rep = np.repeat(bv.reshape(KVH, HD), H // KVH, axis=0).reshape(-1)
    host_const = (Wo @ bv_rep + bo).astype(np.float32)

    out = np.zeros((B, S, D), np.float32)
    for c in range(NCORES):
        b = c // KVH
        out[b] += res.results[c]["resT"].astype(np.float32).T
    out += host_const[None, None, :]
    return out


# revision 44
# speedup vs baseline: 1.1958x; 1.1751x over previous
"""Trainium2 Bass kernel for GQA attention (B=2, S=2048, D=2048, H=16, KVH=4).

Sharding: 8 cores = (batch b in {0,1}) x (kv-group g in {0..3}).
Core c = b*4 + g computes q-heads 4g..4g+3 against kv-head g for batch b,
producing a partial output projection res_partial.T = [e=2048, s=2048] in
bf16; host sums the 4 partials per batch.

v2 design notes (vs the f32r baseline at ~410us):
  - TRN2's PE p-states (0.65 -> 1.2 -> 2.4 GHz, 3us continuous ramp) punish
    every stall; the whole structure exists to keep the PE stream gap-free.
  - All matmuls in bf16 (psums f32): full 1 cyc/row at any free size, lower
    power (less 50% utilization throttling), half the DMA/SBUF footprint.
    Error budget ~1% vs the 2e-2 gate.
  - Weights fully resident in SBUF (no per-chunk wq/wo reloads).
  - RMS-norm partition reduce + broadcasts on the PE (tiny ones-matmuls)
    instead of gpsimd (2.5us/op there).
  - Causal masks are precomputed bf16 0/1 tiles applied by DVE multiply
    (affine_select on gpsimd was 0.6us + heavy semaphore cost).
  - exp on ACT over [128,1024] PSUM groups (amortizes the 352-cycle fixed
    overhead); scores/PV/denominator pipelined one group behind so the exp
    latency never blocks the PE.
  - Softmax denominator stays on the PE (ones-column matmul): cheaper there
    than any partition-reduction alternative.
  - Stage D (o-proj) accumulates in PSUM, copied to bf16 SBUF by DVE (2x
    mode) and DMA'd out as bf16 partials.
"""

import sys

sys.path.insert(0, "/opt/trn_rl_repo")

from contextlib import ExitStack

import numpy as np
import ml_dtypes

import concourse.bass as bass
import concourse.tile as tile
from concourse import bacc, mybir
from concourse import bass_utils

B, S, D = 2, 2048, 2048
H, KVH = 16, 4
HD = 128               # head dim
GQ = 4                 # q heads per core
SL = GQ * HD           # 512: q-head slice width per core
NCORES = 8
SC = S // 512          # 4 s-chunks of 512
KC = D // 128          # 16 d-chunks of 128
ROPE_BASE = 10000.0
EPS = 1.1920929e-07
F32 = mybir.dt.float32
F32R = mybir.dt.float32r
BF16 = mybir.dt.bfloat16
AF = mybir.ActivationFunctionType
BF = ml_dtypes.bfloat16

_COMPILED_NC = None
_LAST_IN_MAPS = None


def _build_body(tc):
    nc = tc.nc
    ctx = ExitStack()
    ctx.enter_context(nc.allow_low_precision(reason="bf16 datapath"))

    xT = nc.dram_tensor("xT", [D, S], BF16, kind="ExternalInput").ap()
    wqT = nc.dram_tensor("wqT", [D, SL], BF16, kind="ExternalInput").ap()
    wkT = nc.dram_tensor("wkT", [D, HD], BF16, kind="ExternalInput").ap()
    wvT = nc.dram_tensor("wvT", [D, HD], BF16, kind="ExternalInput").ap()
    woB = nc.dram_tensor("woB", [SL, D], BF16, kind="ExternalInput").ap()
    csd = nc.dram_tensor("csd", [128, S], BF16, kind="ExternalInput").ap()
    snd = nc.dram_tensor("snd", [128, S], BF16, kind="ExternalInput").ap()
    bqkd = nc.dram_tensor("bqkd", [128, GQ + 1], F32, kind="ExternalInput").ap()
    constd = nc.dram_tensor("constd", [257], F32, kind="ExternalInput").ap()
    sced = nc.dram_tensor("sced", [128, 2 * (GQ + 1)], F32, kind="ExternalInput").ap()
    # bf16 consts: [ident(128) | ones_col(1) | masks(4*512)]
    cbd = nc.dram_tensor("cbd", [128, 128 + 1 + 4 * 512], BF16,
                         kind="ExternalInput").ap()
    resT = nc.dram_tensor("resT", [D, S], BF16, kind="ExternalOutput").ap()

    persist = ctx.enter_context(tc.tile_pool(name="persist", bufs=1))
    xpool = ctx.enter_context(tc.tile_pool(name="xpool", bufs=18))
    bpool = ctx.enter_context(tc.tile_pool(name="bpool", bufs=2))
    rowp = ctx.enter_context(tc.tile_pool(name="rowp", bufs=2))
    expp = ctx.enter_context(tc.tile_pool(name="expp", bufs=3))
    otp = ctx.enter_context(tc.tile_pool(name="otp", bufs=2))
    resp = ctx.enter_context(tc.tile_pool(name="resp", bufs=4))
    vtp = ctx.enter_context(tc.tile_pool(name="vtp", bufs=2))
    # PSUM: exactly 8 banks. psS holds projection-pass pairs, score groups,
    # and stage-D accumulators; psO holds rms broadcasts, v-transposes and
    # attention accumulators; psD holds row vectors + recip broadcasts.
    psS = ctx.enter_context(tc.tile_pool(name="psS", bufs=2, space="PSUM"))
    psO = ctx.enter_context(tc.tile_pool(name="psO", bufs=2, space="PSUM"))
    psD = ctx.enter_context(tc.tile_pool(name="psD", bufs=2, space="PSUM"))

    # ---- persistent tiles ----
    cs_sb = persist.tile([128, S], BF16, name="cs_sb")
    sn_sb = persist.tile([128, S], BF16, name="sn_sb")
    wq_sb = {}
    for kq in range(4):
        for pi in range(2):
            wq_sb[(kq, pi)] = persist.tile([128, 4, 256], BF16,
                                           name=f"wq{kq}_{pi}")
    wk_sb = persist.tile([128, KC, HD], BF16, name="wk_sb")
    wv_sb = persist.tile([128, KC, HD], BF16, name="wv_sb")
    wo_sb = [persist.tile([128, GQ, 256], BF16, name=f"wo{e}") for e in range(8)]
    ident = persist.tile([128, 128], BF16, name="ident")
    ones_col = persist.tile([128, 1], BF16, name="ones_col")
    bqcols = persist.tile([128, GQ + 1], F32, name="bqcols")
    sce = persist.tile([128, 2 * (GQ + 1)], F32, name="sce")

    qfin = [persist.tile([128, S], BF16, name=f"qfin{h}") for h in range(GQ)]
    kfin = persist.tile([128, S], BF16, name="kfin")
    v_sb = [persist.tile([128, HD], BF16, name=f"vsb{i}") for i in range(KC)]

    def emit_const_dmas_small():
        # tiny constants needed by the first stage-B chains (~18us in)
        nc.scalar.dma_start(bqcols, bqkd)
        nc.scalar.dma_start(sce, sced)
        nc.scalar.dma_start(ident, cbd[:, 0:128])
        nc.scalar.dma_start(ones_col, cbd[:, 128:129])

    def emit_const_dmas_mid():
        # rope tables, needed by the first stage-B part2 (~25us in)
        nc.scalar.dma_start(cs_sb, csd)
        nc.sync.dma_start(sn_sb, snd)

    def emit_const_dmas_late():
        # needed by stage D only (~160us in)
        for e in range(8):
            nc.scalar.dma_start(
                wo_sb[e],
                bass.AP(tensor=woB.tensor, offset=e * 256,
                        ap=[[D, 128], [128 * D, GQ], [1, 256]]))

    # ================= Stage B: bias, rms-norm, rope (per [128,512] slice) ===
    # Two-level deferral: part1 (psum reads + sumsq matmul + rsqrt) runs one
    # pass behind the projections; part2 (the rsqrt broadcast matmul + rope)
    # runs two passes behind so the PE never waits on the ACT rsqrt.
    pending_b1 = []
    pending_b2 = []

    def stage_b(et, sc, psum_half):
        """et in 0..3 -> q head et;  et == 4 -> k."""
        is_q = et < GQ
        bias_col = bqcols[:, et: et + 1] if is_q else bqcols[:, GQ: GQ + 1]
        state = {}

        def emit1a():
            q_raw = bpool.tile([128, 512], BF16, tag="qraw", bufs=3,
                               name=f"qraw{et}_{sc}")
            nc.scalar.add(q_raw, psum_half, bias_col)
            sq = bpool.tile([128, 512], BF16, tag="sq", bufs=2,
                            name=f"sq{et}_{sc}")
            nc.scalar.activation(sq, psum_half, AF.Square, bias=bias_col)
            ssq = psD.tile([1, 512], F32, tag="pD", name=f"ssq{et}_{sc}")
            nc.tensor.matmul(ssq, ones_col, sq, start=True, stop=True)
            # rope swap via sbuf->sbuf DMA (sn rows 64..127 hold -sin)
            sw = bpool.tile([128, 512], BF16, tag="sw", bufs=3,
                            name=f"sw{et}_{sc}")
            nc.sync.dma_start(sw[0:64, :], q_raw[64:128, :])
            nc.sync.dma_start(sw[64:128, :], q_raw[0:64, :])
            state["q_raw"] = q_raw
            state["sw"] = sw
            state["ssq"] = ssq

        def emit1b():
            # gain & eps folded: g*rsqrt(ss/HD + eps) == rsqrt(ss*A + B)
            scale_row = bpool.tile([1, 512], F32, tag="srow", bufs=3,
                                   name=f"srow{et}_{sc}")
            nc.scalar.activation(scale_row, state["ssq"],
                                 AF.Abs_reciprocal_sqrt,
                                 bias=sce[0:1, GQ + 1 + et: GQ + 2 + et],
                                 scale=sce[0:1, et: et + 1])
            state["scale_row"] = scale_row

        def emit2():
            # scale broadcast on gpsimd: keeps the PE stream and PSUM pools
            # untouched by the rms chain
            bc = bpool.tile([128, 512], F32, tag="bc", bufs=2,
                            name=f"bc{et}_{sc}")
            nc.gpsimd.partition_broadcast(bc, state["scale_row"])
            t1 = bpool.tile([128, 512], BF16, tag="t1", bufs=2,
                            name=f"t1_{et}_{sc}")
            nc.vector.tensor_mul(t1, state["q_raw"],
                                 cs_sb[:, sc * 512: (sc + 1) * 512])
            t2 = bpool.tile([128, 512], BF16, tag="t2", bufs=2,
                            name=f"t2_{et}_{sc}")
            nc.vector.tensor_mul(t2, state["sw"],
                                 sn_sb[:, sc * 512: (sc + 1) * 512])
            nc.vector.tensor_add(t1, t1, t2)
            dst = qfin[et] if is_q else kfin
            nc.vector.tensor_mul(dst[:, sc * 512: (sc + 1) * 512], t1, bc)

        return emit1a, emit1b, emit2

    def flush_b(final=False):
        while pending_b2:
            pending_b2.pop(0)()
        if final:
            # cluster the rsqrts of the remaining slices so the ACT table
            # set switches once, not per-slice, at the A->C boundary
            staged = list(pending_b1)
            pending_b1.clear()
            for e1a, _, _ in staged:
                e1a()
            for _, e1b, e2 in staged:
                e1b()
                pending_b2.append(e2)
        else:
            while pending_b1:
                e1a, e1b, e2 = pending_b1.pop(0)
                e1a()
                e1b()
                pending_b2.append(e2)

    # ================= Stage A: projections ==================================
    pending_vts = []

    def flush_vts():
        while pending_vts:
            vt, vsc = pending_vts.pop(0)
            for j in range(4):
                stile = vsc * 4 + j
                pst = psO.tile([128, 128], BF16, tag="pO", name=f"pst{stile}")
                nc.tensor.transpose(pst, vt[:, j * 128: (j + 1) * 128], ident)
                nc.vector.tensor_copy(v_sb[stile], pst)

    def emit_xq_dma(xt, sc, kp, eng):
        eng.dma_start(
            xt,
            bass.AP(
                tensor=xT.tensor,
                offset=kp * 256 * S + sc * 512,
                ap=[[S, 128], [128 * S, 2], [1, 512]],
            ),
        )

    xtiles = {}

    def emit_x_dmas(sc, split):
        if sc in xtiles or sc >= SC:
            return
        tl = []
        for kp in range(8):
            xt = xpool.tile([128, 2, 512], BF16, name=f"xq{sc}_{kp}",
                            tag="xq")
            if split:
                # startup: fan the first chunk's tiles over the idle DMA
                # rings so the projection pass is never DMA-paced
                eng = (nc.sync, nc.gpsimd, nc.sync, nc.gpsimd,
                       nc.sync, nc.gpsimd, nc.sync, nc.gpsimd)[kp]
                emit_xq_dma(xt, sc, kp, eng)
            else:
                emit_xq_dma(xt, sc, kp, nc.sync)
            tl.append(xt)
        xtiles[sc] = tl

    for sc in range(SC):
        flush_vts()
        if sc == 0:
            # startup: kv weights first on scalar (kv pass runs first), then
            # x kp4-7, then wq (needed from pass 2 on)
            nc.scalar.dma_start(
                wk_sb, wkT.rearrange("(kc p) h -> p kc h", p=128))
            nc.scalar.dma_start(
                wv_sb, wvT.rearrange("(kc p) h -> p kc h", p=128))
            emit_x_dmas(0, split=True)
            for kq in range(4):
                for pi in range(2):
                    nc.scalar.dma_start(
                        wq_sb[(kq, pi)],
                        bass.AP(tensor=wqT.tensor,
                                offset=kq * 512 * SL + pi * 256,
                                ap=[[SL, 128], [128 * SL, 4], [1, 256]]),
                    )
        xq = xtiles[sc]

        # passes: (k, vT) first, then q-head pairs (0,1), (2,3)
        for pi, grp in enumerate(((GQ, GQ + 1), (0, 1), (2, 3))):
            ps = psS.tile([128, 1024], F32, tag="pS", name=f"pa{sc}_{pi}")
            for kq in range(4):
                for kc4 in range(4):
                    kc = kq * 4 + kc4
                    start = kc == 0
                    stop = kc == KC - 1
                    xsl = xq[kc // 2][:, kc % 2, :]
                    for ei, et in enumerate(grp):
                        if et < GQ:
                            lhsT = wq_sb[(kq, pi - 1)][:, kc4,
                                                       ei * 128: (ei + 1) * 128]
                        elif et == GQ:
                            lhsT = wk_sb[:, kc, :]
                        else:
                            lhsT = wv_sb[:, kc, :]
                        nc.tensor.matmul(ps[:, ei * 512: (ei + 1) * 512],
                                         lhsT, xsl, start=start, stop=stop)
            if pi == 0:
                if sc == 0:
                    emit_const_dmas_small()
                emit_x_dmas(sc + 1, split=False)
                if sc == 0:
                    emit_const_dmas_mid()
                elif sc == 1:
                    emit_const_dmas_late()
            flush_b()
            for ei, et in enumerate(grp):
                half = ps[:, ei * 512: (ei + 1) * 512]
                if et <= GQ:
                    pending_b1.append(stage_b(et, sc, half))
                else:
                    # vT chunk -> bf16 sbuf; PE-transposes deferred to the
                    # next sc so they never stall the pass-boundary stream
                    vt = vtp.tile([128, 512], BF16, tag="vt", bufs=2,
                                  name=f"vt{sc}")
                    nc.scalar.copy(vt, half)
                    pending_vts.append((vt, sc))

    flush_b(final=True)
    flush_b()
    flush_vts()

    # ================= Stage C: attention | Stage D: output proj =============
    # Flat software pipeline over (qc, head, kt-pair-group) work items.
    # PV/denominator drains run ~2 groups behind their exp, interleaved
    # between score matmuls so same-PSUM-bank accumulates stay >=3 apart
    # (back-to-back accumulates into one bank serialize at ~750ns). The
    # normalization chain runs 4-5 items behind (reciprocal on DVE,
    # broadcast on the otherwise-idle gpsimd). Attention psums are
    # allocated lazily at their first drain so pool-buffer reuse never
    # races a not-yet-emitted reader. Stage D for qc is spread as
    # bank-interleaved etg-pairs between the group items of qc+1.

    def groups_for(qc):
        diag = [(4 * qc, 4 * qc + 1), (4 * qc + 2, 4 * qc + 3)]
        off = [(k, k + 1) for k in range(0, 4 * qc, 2)]
        return diag + off

    seq = []
    for qc in range(SC):
        gl = groups_for(qc)
        gitems = [(qc, h, pair, gi == 0, gi == len(gl) - 1)
                  for h in range(GQ) for gi, pair in enumerate(gl)]
        for idx, it in enumerate(gitems):
            seq.append(("g",) + it)
            if qc >= 1 and idx in (5, 7, 9, 11):
                seq.append(("d", qc - 1, idx - 5))
    seq.append(("flushdef",))
    for dp in range(4):
        seq.append(("d", SC - 1, 2 * dp))

    psum_o = {}
    psum_d = {}
    ot_tiles = {}
    pvq = []       # FIFO of half-group PV/denominator drains
    norms = []     # (due_step, fn), dues monotone

    def drain_pv(keep):
        while len(pvq) > keep:
            pvq.pop(0)()

    def flush_norms(s):
        while norms and norms[0][0] <= s:
            norms.pop(0)[1]()

    rf_rows = {}

    def make_pv_half(qc, h, kt, ex, j, st, sp):
        def emit():
            if st:
                psum_o[(qc, h)] = psO.tile([128, 512], F32, tag="pO",
                                           name=f"pso{qc}_{h}")
                psum_d[(qc, h)] = psD.tile([1, 512], F32, tag="pD",
                                           name=f"psd_{qc}_{h}")
            exsl = ex[:, j * 512: (j + 1) * 512]
            nc.tensor.matmul(psum_o[(qc, h)], v_sb[kt], exsl,
                             start=st, stop=sp)
            nc.tensor.matmul(psum_d[(qc, h)], ones_col, exsl,
                             start=st, stop=sp)
        return emit

    def make_norm_a(qc, h):
        def emit():
            rf = rowp.tile([1, 512], F32, tag="rf", bufs=2,
                           name=f"rf{qc}_{h}")
            nc.vector.reciprocal_approx_fast(rf, psum_d[(qc, h)])
            rf_rows[(qc, h)] = rf
        return emit

    def make_norm_b(qc, h):
        def emit():
            rb = bpool.tile([128, 512], F32, tag="rb", bufs=2,
                            name=f"rb{qc}_{h}")
            nc.gpsimd.partition_broadcast(rb, rf_rows[(qc, h)])
            ot = otp.tile([128, 512], BF16, tag=f"ot{h}", name=f"ot{qc}_{h}")
            nc.vector.tensor_mul(ot, psum_o[(qc, h)], rb)
            ot_tiles[(qc, h)] = ot
        return emit

    def emit_stage_d_pair(qc, etg0, tail=False):
        # two etg accumulators live at once; matmuls interleave the four
        # PSUM banks so same-bank accumulates are >=3 apart
        pss = [psS.tile([128, 1024], F32, tag="pS", name=f"psd{qc}_{e}")
               for e in (etg0, etg0 + 1)]
        for h in range(GQ):
            for e2 in range(2):
                for ei, etg in enumerate((etg0, etg0 + 1)):
                    nc.tensor.matmul(
                        pss[ei][:, e2 * 512: (e2 + 1) * 512],
                        wo_sb[etg][:, h, e2 * 128: (e2 + 1) * 128],
                        ot_tiles[(qc, h)],
                        start=(h == 0), stop=(h == GQ - 1),
                    )
        for ei, etg in enumerate((etg0, etg0 + 1)):
            r = resp.tile([128, 2, 512], BF16, tag="res", name=f"r{qc}_{etg}")
            for e2 in range(2):
                # at the tail there are no more exps: use the idle ACT for
                # half the PSUM->SBUF copies so psS bufs recycle faster
                if tail and e2 == 0:
                    nc.scalar.copy(r[:, e2, :],
                                   pss[ei][:, e2 * 512: (e2 + 1) * 512])
                else:
                    nc.vector.tensor_copy(r[:, e2, :],
                                          pss[ei][:, e2 * 512: (e2 + 1) * 512])
            eng = nc.sync
            eng.dma_start(
                bass.AP(tensor=resT.tensor,
                        offset=etg * 2 * 128 * S + qc * 512,
                        ap=[[S, 128], [128 * S, 2], [1, 512]]),
                r,
            )

    step = 0
    for it in seq:
        if it[0] == "g":
            _, qc, h, pair, first, last = it
            flush_norms(step)
            ps = psS.tile([128, 1024], F32, tag="pS",
                          name=f"pss{qc}_{h}_{pair[0]}")
            qsl = qfin[h][:, qc * 512: (qc + 1) * 512]
            ex = expp.tile([128, 1024], BF16, tag="exp",
                           name=f"ex{qc}_{h}_{pair[0]}")
            for j, kt in enumerate(pair):
                nc.tensor.matmul(ps[:, j * 512: (j + 1) * 512],
                                 kfin[:, kt * 128: (kt + 1) * 128], qsl,
                                 start=True, stop=True)
                # keep a 2-group backlog of PV drains between score matmuls
                drain_pv(4 - j)
            nc.scalar.activation(ex, ps, AF.Exp)
            t0 = pair[0] - 4 * qc
            if t0 >= 0:
                # diagonal pair: causal mask (keep q >= k) on gpsimd so the
                # DVE queue never gates the PV chain
                for j in range(2):
                    nc.gpsimd.affine_select(
                        out=ex[:, j * 512: (j + 1) * 512],
                        in_=ex[:, j * 512: (j + 1) * 512],
                        pattern=[[1, 512]],
                        compare_op=mybir.AluOpType.is_ge,
                        fill=0.0,
                        base=-128 * (t0 + j),
                        channel_multiplier=-1,
                    )
            for j, kt in enumerate(pair):
                pvq.append(make_pv_half(qc, h, kt, ex, j,
                                        first and j == 0, last and j == 1))
            if last:
                norms.append((step + 4, make_norm_a(qc, h)))
                norms.append((step + 5, make_norm_b(qc, h)))
            step += 1
        elif it[0] == "d":
            emit_stage_d_pair(it[1], it[2], tail=(it[1] == SC - 1))
        else:  # flushdef
            drain_pv(0)
            flush_norms(10 ** 9)

    ctx.close()


def _build():
    global _COMPILED_NC
    if _COMPILED_NC is not None:
        return _COMPILED_NC
    nc = bacc.Bacc("TRN2", target_bir_lowering=False, debug=False,
                   num_devices=NCORES)
    with tile.TileContext(nc) as tc:
        _build_body(tc)
    nc.compile()
    _COMPILED_NC = nc
    return nc


def _rope_tables():
    inv_freq = 1.0 / (ROPE_BASE ** (np.arange(0, HD, 2, dtype=np.float64) / HD))
    t = np.arange(S, dtype=np.float64)
    freqs = np.outer(t, inv_freq)          # [S, 64]
    cos = np.cos(freqs).T.astype(np.float32)   # [64, S]
    sin = np.sin(freqs).T.astype(np.float32)
    cs = np.concatenate([cos, cos], axis=0)    # [128, S]
    sn = np.concatenate([sin, -sin], axis=0)   # [128, S] (bottom half negated)
    return cs.astype(BF), sn.astype(BF)


def _mask_tiles():
    # maskt[p, t*512 + j] = 1 if causal keeps (k_local=p+128t, q_local=j)
    p = np.arange(128)[:, None]
    j = np.arange(512)[None, :]
    cols = [(j >= p + 128 * t).astype(np.float32) for t in range(4)]
    return np.concatenate(cols, axis=1).astype(BF)


def kernel(x, Wq, bq, Wk, bk, Wv, bv, Wo, bo, q_gain):
    x = np.asarray(x, np.float32)
    Wq = np.asarray(Wq, np.float32)
    bq = np.asarray(bq, np.float32)
    Wk = np.asarray(Wk, np.float32)
    bk = np.asarray(bk, np.float32)
    Wv = np.asarray(Wv, np.float32)
    bv = np.asarray(bv, np.float32)
    Wo = np.asarray(Wo, np.float32)
    bo = np.asarray(bo, np.float32)
    q_gain = np.asarray(q_gain, np.float32)

    cs, sn = _rope_tables()
    const_arr = np.concatenate([
        np.ones(128, np.float32),
        np.full(128, float(HD) ** -0.5, np.float32),
        np.array([EPS], np.float32),
    ])
    cb = np.concatenate([
        np.eye(128, dtype=np.float32).astype(BF),
        np.ones((128, 1), np.float32).astype(BF),
        _mask_tiles(),
    ], axis=1)

    def sced_arr(g):
        gg = np.concatenate([g.astype(np.float64), [HD ** -0.5]])
        a = 1.0 / (HD * gg * gg)
        b = EPS / (gg * gg)
        row = np.concatenate([a, b]).astype(np.float32)   # [2*(GQ+1)]
        return np.ascontiguousarray(np.broadcast_to(row, (128, row.size)))

    in_maps = []
    for c in range(NCORES):
        b, g = divmod(c, KVH)
        sl = slice(g * SL, (g + 1) * SL)
        hs = slice(g * HD, (g + 1) * HD)
        in_maps.append({
            "xT": np.ascontiguousarray(x[b].T.astype(BF)),
            "wqT": np.ascontiguousarray(Wq[sl, :].T.astype(BF)),
            "wkT": np.ascontiguousarray(Wk[hs, :].T.astype(BF)),
            "wvT": np.ascontiguousarray(Wv[hs, :].T.astype(BF)),
            "woB": np.ascontiguousarray(Wo[:, sl].T.astype(BF)),
            "csd": cs, "snd": sn,
            "bqkd": np.ascontiguousarray(np.concatenate(
                [bq[sl].reshape(GQ, HD).T, bk[hs].reshape(1, HD).T], axis=1)),
            "sced": sced_arr(q_gain[g * GQ: (g + 1) * GQ]),
            "constd": const_arr,
            "cbd": cb,
        })

    global _LAST_IN_MAPS
    _LAST_IN_MAPS = in_maps
    nc = _build()
    res = bass_utils.run_bass_kernel_spmd(nc, in_maps, core_ids=list(range(NCORES)))

    # v-bias and o-bias folded on host: attention rows sum to 1, so +bv
    # passes through to O exactly; res += bv_rep @ Wo.T + bo.
    bv_rep = np.repeat(bv.reshape(KVH, HD), H // KVH, axis=0).reshape(-1)
    host_const = (Wo @ bv_rep + bo).astype(np.float32)

    out = np.zeros((B, S, D), np.float32)
    for c in range(NCORES):
        b = c // KVH
        out[b] += res.results[c]["resT"].astype(np.float32).T
    out += host_const[None, None, :]
    return out


# revision 53
# speedup vs baseline: 1.2303x; 1.0289x over previous
"""Trainium2 Bass kernel for GQA attention (B=2, S=2048, D=2048, H=16, KVH=4).

Sharding: 8 cores = (batch b in {0,1}) x (kv-group g in {0..3}).
Core c = b*4 + g computes q-heads 4g..4g+3 against kv-head g for batch b,
producing a partial output projection res_partial.T = [e=2048, s=2048] in
bf16; host sums the 4 partials per batch.

v2 design notes (vs the f32r baseline at ~410us):
  - TRN2's PE p-states (0.65 -> 1.2 -> 2.4 GHz, 3us continuous ramp) punish
    every stall; the whole structure exists to keep the PE stream gap-free.
  - All matmuls in bf16 (psums f32): full 1 cyc/row at any free size, lower
    power (less 50% utilization throttling), half the DMA/SBUF footprint.
    Error budget ~1% vs the 2e-2 gate.
  - Weights fully resident in SBUF (no per-chunk wq/wo reloads).
  - RMS-norm partition reduce + broadcasts on the PE (tiny ones-matmuls)
    instead of gpsimd (2.5us/op there).
  - Causal masks are precomputed bf16 0/1 tiles applied by DVE multiply
    (affine_select on gpsimd was 0.6us + heavy semaphore cost).
  - exp on ACT over [128,1024] PSUM groups (amortizes the 352-cycle fixed
    overhead); scores/PV/denominator pipelined one group behind so the exp
    latency never blocks the PE.
  - Softmax denominator stays on the PE (ones-column matmul): cheaper there
    than any partition-reduction alternative.
  - Stage D (o-proj) accumulates in PSUM, copied to bf16 SBUF by DVE (2x
    mode) and DMA'd out as bf16 partials.
"""

import sys

sys.path.insert(0, "/opt/trn_rl_repo")

from contextlib import ExitStack

import numpy as np
import ml_dtypes

import concourse.bass as bass
import concourse.tile as tile
from concourse import bacc, mybir
from concourse import bass_utils

B, S, D = 2, 2048, 2048
H, KVH = 16, 4
HD = 128               # head dim
GQ = 4                 # q heads per core
SL = GQ * HD           # 512: q-head slice width per core
NCORES = 8
SC = S // 512          # 4 s-chunks of 512
KC = D // 128          # 16 d-chunks of 128
ROPE_BASE = 10000.0
EPS = 1.1920929e-07
F32 = mybir.dt.float32
F32R = mybir.dt.float32r
BF16 = mybir.dt.bfloat16
AF = mybir.ActivationFunctionType
BF = ml_dtypes.bfloat16

_COMPILED_NC = None
_LAST_IN_MAPS = None


def _build_body(tc):
    nc = tc.nc
    ctx = ExitStack()
    ctx.enter_context(nc.allow_low_precision(reason="bf16 datapath"))

    xT = nc.dram_tensor("xT", [D, S], BF16, kind="ExternalInput").ap()
    wqT = nc.dram_tensor("wqT", [D, SL], BF16, kind="ExternalInput").ap()
    wkT = nc.dram_tensor("wkT", [D, HD], BF16, kind="ExternalInput").ap()
    wvT = nc.dram_tensor("wvT", [D, HD], BF16, kind="ExternalInput").ap()
    woB = nc.dram_tensor("woB", [SL, D], BF16, kind="ExternalInput").ap()
    csd = nc.dram_tensor("csd", [128, S], BF16, kind="ExternalInput").ap()
    snd = nc.dram_tensor("snd", [128, S], BF16, kind="ExternalInput").ap()
    bqkd = nc.dram_tensor("bqkd", [128, GQ + 1], F32, kind="ExternalInput").ap()
    constd = nc.dram_tensor("constd", [257], F32, kind="ExternalInput").ap()
    sced = nc.dram_tensor("sced", [128, 2 * (GQ + 1)], F32, kind="ExternalInput").ap()
    # bf16 consts: [ident(128) | ones_col(1) | masks(4*512)]
    cbd = nc.dram_tensor("cbd", [128, 128 + 1 + 4 * 512], BF16,
                         kind="ExternalInput").ap()
    resT = nc.dram_tensor("resT", [D, S], BF16, kind="ExternalOutput").ap()

    persist = ctx.enter_context(tc.tile_pool(name="persist", bufs=1))
    xpool = ctx.enter_context(tc.tile_pool(name="xpool", bufs=18))
    bpool = ctx.enter_context(tc.tile_pool(name="bpool", bufs=2))
    rowp = ctx.enter_context(tc.tile_pool(name="rowp", bufs=2))
    expp = ctx.enter_context(tc.tile_pool(name="expp", bufs=4))
    otp = ctx.enter_context(tc.tile_pool(name="otp", bufs=2))
    resp = ctx.enter_context(tc.tile_pool(name="resp", bufs=4))
    vtp = ctx.enter_context(tc.tile_pool(name="vtp", bufs=2))
    # PSUM: exactly 8 banks. psS holds projection-pass pairs, score groups,
    # and stage-D accumulators; psO holds rms broadcasts, v-transposes and
    # attention accumulators; psD holds row vectors + recip broadcasts.
    psS = ctx.enter_context(tc.tile_pool(name="psS", bufs=2, space="PSUM"))
    psO = ctx.enter_context(tc.tile_pool(name="psO", bufs=2, space="PSUM"))
    psD = ctx.enter_context(tc.tile_pool(name="psD", bufs=2, space="PSUM"))

    # ---- persistent tiles ----
    cs_sb = persist.tile([128, S], BF16, name="cs_sb")
    sn_sb = persist.tile([128, S], BF16, name="sn_sb")
    wq_sb = {}
    for kq in range(4):
        for pi in range(2):
            wq_sb[(kq, pi)] = persist.tile([128, 4, 256], BF16,
                                           name=f"wq{kq}_{pi}")
    wk_sb = persist.tile([128, KC, HD], BF16, name="wk_sb")
    wv_sb = persist.tile([128, KC, HD], BF16, name="wv_sb")
    wo_sb = [persist.tile([128, GQ, 256], BF16, name=f"wo{e}") for e in range(8)]
    ident = persist.tile([128, 128], BF16, name="ident")
    ones_col = persist.tile([128, 1], BF16, name="ones_col")
    bqcols = persist.tile([128, GQ + 1], F32, name="bqcols")
    sce = persist.tile([128, 2 * (GQ + 1)], F32, name="sce")

    qfin = [persist.tile([128, S], BF16, name=f"qfin{h}") for h in range(GQ)]
    kfin = persist.tile([128, S], BF16, name="kfin")
    v_sb = [persist.tile([128, HD], BF16, name=f"vsb{i}") for i in range(KC)]

    def emit_const_dmas_small():
        # tiny constants needed by the first stage-B chains (~18us in)
        nc.scalar.dma_start(bqcols, bqkd)
        nc.scalar.dma_start(sce, sced)
        nc.scalar.dma_start(ident, cbd[:, 0:128])
        nc.scalar.dma_start(ones_col, cbd[:, 128:129])

    def emit_const_dmas_mid():
        # rope tables, needed by the first stage-B part2 (~25us in)
        nc.scalar.dma_start(cs_sb, csd)
        nc.sync.dma_start(sn_sb, snd)

    def emit_const_dmas_late():
        # needed by stage D only (~160us in)
        for e in range(8):
            nc.scalar.dma_start(
                wo_sb[e],
                bass.AP(tensor=woB.tensor, offset=e * 256,
                        ap=[[D, 128], [128 * D, GQ], [1, 256]]))

    # ================= Stage B: bias, rms-norm, rope (per [128,512] slice) ===
    # Two-level deferral: part1 (psum reads + sumsq matmul + rsqrt) runs one
    # pass behind the projections; part2 (the rsqrt broadcast matmul + rope)
    # runs two passes behind so the PE never waits on the ACT rsqrt.
    pending_b1 = []
    pending_b2 = []

    def stage_b(et, sc, psum_half):
        """et in 0..3 -> q head et;  et == 4 -> k."""
        is_q = et < GQ
        bias_col = bqcols[:, et: et + 1] if is_q else bqcols[:, GQ: GQ + 1]
        state = {}

        def emit1a():
            q_raw = bpool.tile([128, 512], BF16, tag="qraw", bufs=3,
                               name=f"qraw{et}_{sc}")
            nc.scalar.add(q_raw, psum_half, bias_col)
            sq = bpool.tile([128, 512], BF16, tag="sq", bufs=2,
                            name=f"sq{et}_{sc}")
            nc.scalar.activation(sq, psum_half, AF.Square, bias=bias_col)
            ssq = psD.tile([1, 512], F32, tag="pD", name=f"ssq{et}_{sc}")
            nc.tensor.matmul(ssq, ones_col, sq, start=True, stop=True)
            # rope swap via sbuf->sbuf DMA (sn rows 64..127 hold -sin)
            sw = bpool.tile([128, 512], BF16, tag="sw", bufs=3,
                            name=f"sw{et}_{sc}")
            nc.sync.dma_start(sw[0:64, :], q_raw[64:128, :])
            nc.sync.dma_start(sw[64:128, :], q_raw[0:64, :])
            state["q_raw"] = q_raw
            state["sw"] = sw
            state["ssq"] = ssq

        def emit1b():
            # gain & eps folded: g*rsqrt(ss/HD + eps) == rsqrt(ss*A + B)
            scale_row = bpool.tile([1, 512], F32, tag="srow", bufs=3,
                                   name=f"srow{et}_{sc}")
            nc.scalar.activation(scale_row, state["ssq"],
                                 AF.Abs_reciprocal_sqrt,
                                 bias=sce[0:1, GQ + 1 + et: GQ + 2 + et],
                                 scale=sce[0:1, et: et + 1])
            state["scale_row"] = scale_row

        def emit2():
            # scale broadcast on gpsimd: keeps the PE stream and PSUM pools
            # untouched by the rms chain
            bc = bpool.tile([128, 512], F32, tag="bc", bufs=2,
                            name=f"bc{et}_{sc}")
            nc.gpsimd.partition_broadcast(bc, state["scale_row"])
            t1 = bpool.tile([128, 512], BF16, tag="t1", bufs=2,
                            name=f"t1_{et}_{sc}")
            nc.vector.tensor_mul(t1, state["q_raw"],
                                 cs_sb[:, sc * 512: (sc + 1) * 512])
            t2 = bpool.tile([128, 512], BF16, tag="t2", bufs=2,
                            name=f"t2_{et}_{sc}")
            nc.vector.tensor_mul(t2, state["sw"],
                                 sn_sb[:, sc * 512: (sc + 1) * 512])
            nc.vector.tensor_add(t1, t1, t2)
            dst = qfin[et] if is_q else kfin
            nc.vector.tensor_mul(dst[:, sc * 512: (sc + 1) * 512], t1, bc)

        return emit1a, emit1b, emit2

    def flush_b(final=False):
        while pending_b2:
            pending_b2.pop(0)()
        if final:
            # cluster the rsqrts of the remaining slices so the ACT table
            # set switches once, not per-slice, at the A->C boundary
            staged = list(pending_b1)
            pending_b1.clear()
            for e1a, _, _ in staged:
                e1a()
            for _, e1b, e2 in staged:
                e1b()
                pending_b2.append(e2)
        else:
            while pending_b1:
                e1a, e1b, e2 = pending_b1.pop(0)
                e1a()
                e1b()
                pending_b2.append(e2)

    # ================= Stage A: projections ==================================
    pending_vts = []

    def flush_vts():
        while pending_vts:
            vt, vsc = pending_vts.pop(0)
            for j in range(4):
                stile = vsc * 4 + j
                pst = psO.tile([128, 128], BF16, tag="pO", name=f"pst{stile}")
                nc.tensor.transpose(pst, vt[:, j * 128: (j + 1) * 128], ident)
                nc.vector.tensor_copy(v_sb[stile], pst)

    def emit_xq_dma(xt, sc, kp, eng):
        eng.dma_start(
            xt,
            bass.AP(
                tensor=xT.tensor,
                offset=kp * 256 * S + sc * 512,
                ap=[[S, 128], [128 * S, 2], [1, 512]],
            ),
        )

    xtiles = {}

    def emit_x_dmas(sc, split):
        if sc in xtiles or sc >= SC:
            return
        tl = []
        for kp in range(8):
            xt = xpool.tile([128, 2, 512], BF16, name=f"xq{sc}_{kp}",
                            tag="xq")
            if split:
                # startup: alternate the two hardware DGE rings so the
                # first projection pass is never DMA-paced
                eng = nc.sync if kp % 2 == 0 else nc.scalar
                emit_xq_dma(xt, sc, kp, eng)
            else:
                emit_xq_dma(xt, sc, kp, nc.sync)
            tl.append(xt)
        xtiles[sc] = tl

    for sc in range(SC):
        flush_vts()
        if sc == 0:
            # startup: kv weights first on scalar (kv pass runs first), then
            # x kp4-7, then wq (needed from pass 2 on)
            nc.scalar.dma_start(
                wk_sb, wkT.rearrange("(kc p) h -> p kc h", p=128))
            nc.scalar.dma_start(
                wv_sb, wvT.rearrange("(kc p) h -> p kc h", p=128))
            emit_x_dmas(0, split=True)
            for kq in range(4):
                for pi in range(2):
                    nc.scalar.dma_start(
                        wq_sb[(kq, pi)],
                        bass.AP(tensor=wqT.tensor,
                                offset=kq * 512 * SL + pi * 256,
                                ap=[[SL, 128], [128 * SL, 4], [1, 256]]),
                    )
        xq = xtiles[sc]

        # passes: (k, vT) first, then q-head pairs (0,1), (2,3)
        for pi, grp in enumerate(((GQ, GQ + 1), (0, 1), (2, 3))):
            ps = psS.tile([128, 1024], F32, tag="pS", name=f"pa{sc}_{pi}")
            for kq in range(4):
                for kc4 in range(4):
                    kc = kq * 4 + kc4
                    start = kc == 0
                    stop = kc == KC - 1
                    xsl = xq[kc // 2][:, kc % 2, :]
                    for ei, et in enumerate(grp):
                        if et < GQ:
                            lhsT = wq_sb[(kq, pi - 1)][:, kc4,
                                                       ei * 128: (ei + 1) * 128]
                        elif et == GQ:
                            lhsT = wk_sb[:, kc, :]
                        else:
                            lhsT = wv_sb[:, kc, :]
                        nc.tensor.matmul(ps[:, ei * 512: (ei + 1) * 512],
                                         lhsT, xsl, start=start, stop=stop)
            if pi == 0:
                if sc == 0:
                    emit_const_dmas_small()
                emit_x_dmas(sc + 1, split=False)
                if sc == 0:
                    emit_const_dmas_mid()
                elif sc == 1:
                    emit_const_dmas_late()
            flush_b()
            for ei, et in enumerate(grp):
                half = ps[:, ei * 512: (ei + 1) * 512]
                if et <= GQ:
                    pending_b1.append(stage_b(et, sc, half))
                else:
                    # vT chunk -> bf16 sbuf; PE-transposes deferred to the
                    # next sc so they never stall the pass-boundary stream
                    vt = vtp.tile([128, 512], BF16, tag="vt", bufs=2,
                                  name=f"vt{sc}")
                    nc.scalar.copy(vt, half)
                    pending_vts.append((vt, sc))

    flush_b(final=True)
    flush_b()
    flush_vts()

    # ================= Stage C: attention | Stage D: output proj =============
    # Flat software pipeline over (qc, head, kt-pair-group) work items.
    # PV/denominator drains run ~2 groups behind their exp, interleaved
    # between score matmuls so same-PSUM-bank accumulates stay >=3 apart
    # (back-to-back accumulates into one bank serialize at ~750ns). The
    # normalization chain runs 4-5 items behind (reciprocal on DVE,
    # broadcast on the otherwise-idle gpsimd). Attention psums are
    # allocated lazily at their first drain so pool-buffer reuse never
    # races a not-yet-emitted reader. Stage D for qc is spread as
    # bank-interleaved etg-pairs between the group items of qc+1.

    def groups_for(qc):
        diag = [(4 * qc, 4 * qc + 1), (4 * qc + 2, 4 * qc + 3)]
        off = [(k, k + 1) for k in range(0, 4 * qc, 2)]
        return diag + off

    seq = []
    for qc in range(SC):
        gl = groups_for(qc)
        gitems = [(qc, h, pair, gi == 0, gi == len(gl) - 1)
                  for h in range(GQ) for gi, pair in enumerate(gl)]
        for idx, it in enumerate(gitems):
            seq.append(("g",) + it)
            if qc >= 1 and idx in (5, 7, 9, 11):
                seq.append(("d", qc - 1, idx - 5))
    seq.append(("flushdef",))
    seq.append(("dtail",))

    psum_o = {}
    psum_d = {}
    ot_tiles = {}
    pvq = []       # FIFO of half-group PV/denominator drains
    norms = []     # (due_step, fn), dues monotone

    def drain_pv(keep):
        while len(pvq) > keep:
            pvq.pop(0)()

    def flush_norms(s):
        while norms and norms[0][0] <= s:
            norms.pop(0)[1]()

    rf_rows = {}

    def make_pv_half(qc, h, kt, ex, j, st, sp):
        def emit():
            if st:
                psum_o[(qc, h)] = psO.tile([128, 512], F32, tag="pO",
                                           name=f"pso{qc}_{h}")
                psum_d[(qc, h)] = psD.tile([1, 512], F32, tag="pD",
                                           name=f"psd_{qc}_{h}")
            exsl = ex[:, j * 512: (j + 1) * 512]
            nc.tensor.matmul(psum_o[(qc, h)], v_sb[kt], exsl,
                             start=st, stop=sp)
            nc.tensor.matmul(psum_d[(qc, h)], ones_col, exsl,
                             start=st, stop=sp)
            if sp:
                # normalization chain emitted inline right after the stop
                # drain: no PE ops in it, and this guarantees it precedes
                # any pool-buffer reuse by later heads
                make_norm_a(qc, h)()
                make_norm_b(qc, h)()
        return emit

    def make_norm_a(qc, h):
        def emit():
            rf = rowp.tile([1, 512], F32, tag="rf", bufs=2,
                           name=f"rf{qc}_{h}")
            nc.vector.reciprocal_approx_fast(rf, psum_d[(qc, h)])
            rf_rows[(qc, h)] = rf
        return emit

    def make_norm_b(qc, h):
        def emit():
            rb = bpool.tile([128, 512], F32, tag="rb", bufs=2,
                            name=f"rb{qc}_{h}")
            nc.gpsimd.partition_broadcast(rb, rf_rows[(qc, h)])
            ot = otp.tile([128, 512], BF16, tag=f"ot{h}", name=f"ot{qc}_{h}")
            nc.vector.tensor_mul(ot, psum_o[(qc, h)], rb)
            ot_tiles[(qc, h)] = ot
        return emit

    def emit_stage_d_pair(qc, etg0, tail=False):
        # two etg accumulators live at once; matmuls interleave the four
        # PSUM banks so same-bank accumulates are >=3 apart
        pss = [psS.tile([128, 1024], F32, tag="pS", name=f"psd{qc}_{e}")
               for e in (etg0, etg0 + 1)]
        for h in range(GQ):
            for e2 in range(2):
                for ei, etg in enumerate((etg0, etg0 + 1)):
                    nc.tensor.matmul(
                        pss[ei][:, e2 * 512: (e2 + 1) * 512],
                        wo_sb[etg][:, h, e2 * 128: (e2 + 1) * 128],
                        ot_tiles[(qc, h)],
                        start=(h == 0), stop=(h == GQ - 1),
                    )
        for ei, etg in enumerate((etg0, etg0 + 1)):
            r = resp.tile([128, 2, 512], BF16, tag="res", name=f"r{qc}_{etg}")
            for e2 in range(2):
                # at the tail there are no more exps: use the idle ACT for
                # half the PSUM->SBUF copies so psS bufs recycle faster
                if tail and e2 == 0:
                    nc.scalar.copy(r[:, e2, :],
                                   pss[ei][:, e2 * 512: (e2 + 1) * 512])
                else:
                    nc.vector.tensor_copy(r[:, e2, :],
                                          pss[ei][:, e2 * 512: (e2 + 1) * 512])
            eng = nc.sync
            eng.dma_start(
                bass.AP(tensor=resT.tensor,
                        offset=etg * 2 * 128 * S + qc * 512,
                        ap=[[S, 128], [128 * S, 2], [1, 512]]),
                r,
            )

    def emit_stage_d_tail():
        # last qc's o-proj has nothing left to interleave with: use all 8
        # PSUM banks (psS + the freed psO/psD) for 4 concurrent etg
        # accumulators per wave, copies split across ACT and DVE
        qc = SC - 1
        for wave in range(2):
            es = [4 * wave + k for k in range(4)]
            accs = {}
            for k in range(2):
                t = psS.tile([128, 1024], F32, tag="pS",
                             name=f"pst{qc}_{es[k]}")
                accs[es[k]] = [t[:, 0:512], t[:, 512:1024]]
            accs[es[2]] = [psO.tile([128, 512], F32, tag="pO",
                                    name=f"pto{qc}_{e2}") for e2 in range(2)]
            accs[es[3]] = [psD.tile([128, 512], F32, tag="pD",
                                    name=f"ptd{qc}_{e2}") for e2 in range(2)]
            for h in range(GQ):
                for e2 in range(2):
                    for e in es:
                        nc.tensor.matmul(
                            accs[e][e2],
                            wo_sb[e][:, h, e2 * 128: (e2 + 1) * 128],
                            ot_tiles[(qc, h)],
                            start=(h == 0), stop=(h == GQ - 1),
                        )
            for e in es:
                r = resp.tile([128, 2, 512], BF16, tag="res",
                              name=f"r{qc}_{e}")
                nc.scalar.copy(r[:, 0, :], accs[e][0])
                nc.vector.tensor_copy(r[:, 1, :], accs[e][1])
                nc.sync.dma_start(
                    bass.AP(tensor=resT.tensor,
                            offset=e * 2 * 128 * S + qc * 512,
                            ap=[[S, 128], [128 * S, 2], [1, 512]]),
                    r,
                )

    step = 0
    for it in seq:
        if it[0] == "g":
            _, qc, h, pair, first, last = it
            flush_norms(step)
            ps = psS.tile([128, 1024], F32, tag="pS",
                          name=f"pss{qc}_{h}_{pair[0]}")
            qsl = qfin[h][:, qc * 512: (qc + 1) * 512]
            ex = expp.tile([128, 1024], BF16, tag="exp",
                           name=f"ex{qc}_{h}_{pair[0]}")
            for j, kt in enumerate(pair):
                nc.tensor.matmul(ps[:, j * 512: (j + 1) * 512],
                                 kfin[:, kt * 128: (kt + 1) * 128], qsl,
                                 start=True, stop=True)
                # keep a 3-group backlog of PV drains between score matmuls
                # (gives the gpsimd causal-mask selects ~5us of slack)
                drain_pv(6 - j)
            nc.scalar.activation(ex, ps, AF.Exp)
            t0 = pair[0] - 4 * qc
            if t0 >= 0:
                # diagonal pair: causal mask (keep q >= k) on gpsimd so the
                # DVE queue never gates the PV chain
                for j in range(2):
                    nc.gpsimd.affine_select(
                        out=ex[:, j * 512: (j + 1) * 512],
                        in_=ex[:, j * 512: (j + 1) * 512],
                        pattern=[[1, 512]],
                        compare_op=mybir.AluOpType.is_ge,
                        fill=0.0,
                        base=-128 * (t0 + j),
                        channel_multiplier=-1,
                    )
            for j, kt in enumerate(pair):
                pvq.append(make_pv_half(qc, h, kt, ex, j,
                                        first and j == 0, last and j == 1))
            step += 1
        elif it[0] == "d":
            emit_stage_d_pair(it[1], it[2])
        elif it[0] == "dtail":
            emit_stage_d_tail()
        else:  # flushdef
            drain_pv(0)
            flush_norms(10 ** 9)

    ctx.close()


def _build():
    global _COMPILED_NC
    if _COMPILED_NC is not None:
        return _COMPILED_NC
    nc = bacc.Bacc("TRN2", target_bir_lowering=False, debug=False,
                   num_devices=NCORES)
    with tile.TileContext(nc) as tc:
        _build_body(tc)
    nc.compile()
    _COMPILED_NC = nc
    return nc


def _rope_tables():
    inv_freq = 1.0 / (ROPE_BASE ** (np.arange(0, HD, 2, dtype=np.float64) / HD))
    t = np.arange(S, dtype=np.float64)
    freqs = np.outer(t, inv_freq)          # [S, 64]
    cos = np.cos(freqs).T.astype(np.float32)   # [64, S]
    sin = np.sin(freqs).T.astype(np.float32)
    cs = np.concatenate([cos, cos], axis=0)    # [128, S]
    sn = np.concatenate([sin, -sin], axis=0)   # [128, S] (bottom half negated)
    return cs.astype(BF), sn.astype(BF)


def _mask_tiles():
    # maskt[p, t*512 + j] = 1 if causal keeps (k_local=p+128t, q_local=j)
    p = np.arange(128)[:, None]
    j = np.arange(512)[None, :]
    cols = [(j >= p + 128 * t).astype(np.float32) for t in range(4)]
    return np.concatenate(cols, axis=1).astype(BF)


def kernel(x, Wq, bq, Wk, bk, Wv, bv, Wo, bo, q_gain):
    x = np.asarray(x, np.float32)
    Wq = np.asarray(Wq, np.float32)
    bq = np.asarray(bq, np.float32)
    Wk = np.asarray(Wk, np.float32)
    bk = np.asarray(bk, np.float32)
    Wv = np.asarray(Wv, np.float32)
    bv = np.asarray(bv, np.float32)
    Wo = np.asarray(Wo, np.float32)
    bo = np.asarray(bo, np.float32)
    q_gain = np.asarray(q_gain, np.float32)

    cs, sn = _rope_tables()
    const_arr = np.concatenate([
        np.ones(128, np.float32),
        np.full(128, float(HD) ** -0.5, np.float32),
        np.array([EPS], np.float32),
    ])
    cb = np.concatenate([
        np.eye(128, dtype=np.float32).astype(BF),
        np.ones((128, 1), np.float32).astype(BF),
        _mask_tiles(),
    ], axis=1)

    def sced_arr(g):
        gg = np.concatenate([g.astype(np.float64), [HD ** -0.5]])
        a = 1.0 / (HD * gg * gg)
        b = EPS / (gg * gg)
        row = np.concatenate([a, b]).astype(np.float32)   # [2*(GQ+1)]
        return np.ascontiguousarray(np.broadcast_to(row, (128, row.size)))

    in_maps = []
    for c in range(NCORES):
        b, g = divmod(c, KVH)
        sl = slice(g * SL, (g + 1) * SL)
        hs = slice(g * HD, (g + 1) * HD)
        in_maps.append({
            "xT": np.ascontiguousarray(x[b].T.astype(BF)),
            "wqT": np.ascontiguousarray(Wq[sl, :].T.astype(BF)),
            "wkT": np.ascontiguousarray(Wk[hs, :].T.astype(BF)),
            "wvT": np.ascontiguousarray(Wv[hs, :].T.astype(BF)),
            "woB": np.ascontiguousarray(Wo[:, sl].T.astype(BF)),
            "csd": cs, "snd": sn,
            "bqkd": np.ascontiguousarray(np.concatenate(
                [bq[sl].reshape(GQ, HD).T, bk[hs].reshape(1, HD).T], axis=1)),
            "sced": sced_arr(q_gain[g * GQ: (g + 1) * GQ]),
            "constd": const_arr,
            "cbd": cb,
        })

    global _LAST_IN_MAPS
    _LAST_IN_MAPS = in_maps
    nc = _build()
    res = bass_utils.run_bass_kernel_spmd(nc, in_maps, core_ids=list(range(NCORES)))

    # v-bias and o-bias folded on host: attention rows sum to 1, so +bv
    # passes through to O exactly; res += bv_rep @ Wo.T + bo.
    bv_rep = np.repeat(bv.reshape(KVH, HD), H // KVH, axis=0).reshape(-1)
    host_const = (Wo @ bv_rep + bo).astype(np.float32)

    out = np.zeros((B, S, D), np.float32)
    for c in range(NCORES):
        b = c // KVH
        out[b] += res.results[c]["resT"].astype(np.float32).T
    out += host_const[None, None, :]
    return out
